# revision 8
# baseline (speedup 1.0000x reference)
"""Cross-channel multi-head attention on 8 Trainium2 NeuronCores.

Sharding: data-parallel over the batch axis. bs2=16 sequences form bs=8
(batch, 2-channel) pairs; each core handles one pair fully locally
(cross-channel attention couples only the two channels of the same batch
element), so no collectives are needed.

Per core (T=2048 tokens = 2 channels x 1024 patches, D=1024, H=8 heads,
dk=128; heads 0..5 attend to the other channel's K/V, heads 6..7 to the
same channel):
  1. V = x @ Wv + bv in natural [T, D] layout (phase A), Qt/Kt = per-head
     [dk, T] projections (phase B) -- all matmul streams 512 wide so the
     PE's per-matmul LDWEIGHTS hides under the previous multiply.
  2. Per (head, channel) unit: S^T chunks = Kt-chunk^T x Qt (psum
     [128, 1024] spanning 2 banks, halves as separate matmul groups);
     P^T = exp(S^T/sqrt(dk)) via one [128,1024] scalar ACTIVATE per chunk.
  3. attn@V with V as the STATIONARY operand and P^T streaming 512 wide:
     Z^T[dk, n] accumulates directly in psum -- no PE transposes, no
     ones-column. Softmax denominators: DVE pair-add tree over the 8 P^T
     chunks (bf16), then an all-ones [128,128] stationary matmul whose
     output is the partition sum REPLICATED across all 128 partitions
     (f32 accumulate), DVE reciprocal psum->sbuf; the normalize then
     multiplies along the free dim in the psum->Zt copy.
  4. out = Zt-chunks^T @ Wo + bo, stored bf16 (host upcasts to f32).

All matmuls bf16 with f32 PSUM accumulation. Denominator tree in bf16
(values ~1e2..1e4, well within range; adds ~0.3% rel err, total ~0.8%
vs the 2e-2 gate). The host pre-transposes/casts x to bf16 [D, T] per
core and casts the weights to bf16.
"""

import sys

if "/opt/trn_rl_repo" not in sys.path:
    sys.path.insert(0, "/opt/trn_rl_repo")

import numpy as np
import ml_dtypes

import concourse.bass as bass
import concourse.bass_isa as bass_isa
import concourse.tile as tile
from concourse import mybir
from concourse.bass_utils import run_bass_kernel_spmd

# Walrus in this container rejects >1 wait condition on TPB_CTRL ops
# (Tile's kernel-tail drain carries one per active proc). Split them.
import os

_here = os.path.dirname(os.path.abspath(__file__))
if _here not in sys.path:
    sys.path.insert(0, _here)
try:
    import bir_legalize
except ImportError:  # graded in a bare dir: fall back to inline copy
    bir_legalize = None

N = 1024  # patches per channel
D = 1024
H = 8
DK = 128
N_CROSS = 6
T = 2 * N  # tokens per core (2 channels of one batch element)
P = 128
KO = D // P  # 8 outer chunks of the 1024-wide dims
TC = T // P  # 16 token chunks
BF = mybir.dt.bfloat16
F32 = mybir.dt.float32
SCALE = 1.0 / float(np.sqrt(DK))
EXP = mybir.ActivationFunctionType.Exp
ADD = mybir.AluOpType.add
MULT = mybir.AluOpType.mult

_CACHE = {}


def _legalize_install():
    if bir_legalize is not None:
        bir_legalize.install()
        return
    # Inline fallback (kernel.py must be self-contained when graded).
    import json
    import concourse.bass2jax as bass2jax
    from concourse.bass_utils import compile_bir_kernel as _orig

    if getattr(bass2jax.compile_bir_kernel, "_legalized", False):
        return

    OPCODE_MAX = {}
    SKIP = set()

    def _legalize(bir_json):
        d = json.loads(bir_json)
        changed = False
        for fn in d.get("functions", []):
            for bb in fn.get("blocks") or fn.get("basicblocks") or []:
                out = []
                for inst in bb.get("instructions", []):
                    sync = inst.get("sync_info") or {}
                    waits = sync.get("on_wait") or []
                    cap = OPCODE_MAX.get(inst.get("opcode"), 1)
                    if len(waits) > cap and inst.get("opcode") not in SKIP:
                        extra, keep = waits[:-cap], waits[-cap:]
                        for i, w in enumerate(extra):
                            out.append(
                                {
                                    "debug": inst.get("debug", 0),
                                    "engine": inst["engine"],
                                    "ins": [],
                                    "outs": [],
                                    "is_reset_sema": False,
                                    "name": f"{inst['name']}-sw{i}",
                                    "opcode": "Drain",
                                    "sync_info": {"on_update": [], "on_wait": [w]},
                                }
                            )
                        sync["on_wait"] = keep
                        inst["sync_info"] = sync
                        changed = True
                    out.append(inst)
                bb["instructions"] = out
        return json.dumps(d).encode() if changed else bir_json

    def compile_bir_kernel(bir_json, tmpdir, neff_name="file.neff"):
        return _orig(_legalize(bir_json), tmpdir, neff_name)

    compile_bir_kernel._legalized = True
    bass2jax.compile_bir_kernel = compile_bir_kernel


def _bcast_rows(ap, p):
    """Replicate a 1-D DRAM AP across p partitions (stride-0 partition dim)."""
    return bass.AP(tensor=ap.tensor, offset=ap.offset, ap=[[0, p], *ap.ap])


def _build():
    nc = bass.Bass()

    xt_d = nc.dram_tensor("xt", [D, T], BF, kind="ExternalInput").ap()
    wq_d = nc.dram_tensor("wq", [D, D], BF, kind="ExternalInput").ap()
    wk_d = nc.dram_tensor("wk", [D, D], BF, kind="ExternalInput").ap()
    wv_d = nc.dram_tensor("wv", [D, D], BF, kind="ExternalInput").ap()
    wo_d = nc.dram_tensor("wo", [D, D], BF, kind="ExternalInput").ap()
    bq_d = nc.dram_tensor("bq", [D], F32, kind="ExternalInput").ap()
    bk_d = nc.dram_tensor("bk", [D], F32, kind="ExternalInput").ap()
    bv_d = nc.dram_tensor("bv", [D], F32, kind="ExternalInput").ap()
    bo_d = nc.dram_tensor("bo", [D], F32, kind="ExternalInput").ap()
    out_d = nc.dram_tensor("out", [T, D], BF, kind="ExternalOutput").ap()

    xt_r = xt_d.rearrange("(o p) t -> p o t", p=P)
    wq_r = wq_d.rearrange("(o p) f -> p o f", p=P)
    wk_r = wk_d.rearrange("(o p) f -> p o f", p=P)
    wv_r = wv_d.rearrange("(o p) f -> p o f", p=P)
    wo_r = wo_d.rearrange("(o p) f -> p o f", p=P)

    with tile.TileContext(nc) as tc:
        with (
            tc.tile_pool(name="consts", bufs=1) as consts,
            tc.tile_pool(name="big", bufs=1) as big,
            tc.tile_pool(name="xt_w", bufs=1) as xt_w,
            tc.tile_pool(name="ps_a", bufs=2, space="PSUM") as ps_a,
            tc.tile_pool(name="ps_z", bufs=2, space="PSUM") as ps_z,
        ):
            Vg = big.tile([P, TC, H, DK], BF)  # V natural, per token-chunk/head
            Zt = big.tile([P, H, T], BF)  # attention out, [dout, T]
            bq_p = consts.tile([P, KO], F32)
            bk_p = consts.tile([P, KO], F32)
            bo_r = consts.tile([P, D], F32)

            # ---- phase A: V projection ----
            with tc.tile_pool(name="wv_pool", bufs=1) as wv_pool:
                Xt = xt_w.tile([P, KO, T], BF)
                Wq = xt_w.tile([P, KO, D], BF)
                Wk = xt_w.tile([P, KO, D], BF)
                Wv = wv_pool.tile([P, KO, D], BF)
                bv_r = wv_pool.tile([P, D], F32)

                # DMA issue order is the startup critical path. sync carries
                # the first-half Xt k-chunks (consumed k-ascending by batch
                # 0/1), scalar carries Wv then biases then Wq/Wk, gpsimd
                # carries the second-half Xt (batches 2/3, prefetched).
                nc.sync.dma_start(Xt[:, 0, 0:512], xt_r[:, 0, 0:512])
                nc.scalar.dma_start(Wv[:, 0, :], wv_r[:, 0, :])
                for k in range(1, KO):
                    nc.sync.dma_start(Xt[:, k, 0:512], xt_r[:, k, 0:512])
                    nc.scalar.dma_start(Wv[:, k, :], wv_r[:, k, :])
                for k in range(KO):
                    nc.sync.dma_start(Xt[:, k, 512:1024], xt_r[:, k, 512:1024])
                    nc.gpsimd.dma_start(Xt[:, k, 1024:2048], xt_r[:, k, 1024:2048])
                nc.scalar.dma_start(bq_p[:], bq_d.rearrange("(o p) -> p o", p=P))
                nc.scalar.dma_start(bk_p[:], bk_d.rearrange("(o p) -> p o", p=P))
                nc.scalar.dma_start(bv_r[:], _bcast_rows(bv_d, P))
                for k in range(KO):
                    nc.scalar.dma_start(Wq[:, k, :], wq_r[:, k, :])
                    nc.scalar.dma_start(Wk[:, k, :], wk_r[:, k, :])
                nc.scalar.dma_start(bo_r[:], _bcast_rows(bo_d, P))

                # Warm the PE HAM clock gate with throwaway accumulating
                # matmuls so the first real matmuls run at 2.4 GHz; sized to
                # roughly cover the first-input DMA window.
                warm_in = consts.tile([P, P], BF)
                nc.vector.memset(warm_in[:], 0.0)
                warm_rhs = consts.tile([P, 512], BF)
                nc.vector.memset(warm_rhs[:], 0.0)
                ones128 = consts.tile([P, P], BF)
                nc.vector.memset(ones128[:], 1.0)
                for g in range(2):
                    wps = ps_a.tile([P, D], F32, tag="a", name=f"warm_{g}")
                    for k in range(8):
                        nc.tensor.matmul(
                            wps[:, 0:512],
                            warm_in[:],
                            warm_rhs[:],
                            start=(k == 0),
                            stop=(k == 7),
                        )

                # V natural = Xt-chunk.T @ Wv, 4 two-bank psum groups in
                # flight (8 banks) so the PE has backlog while inputs stream.
                for base in range(0, TC, 4):
                    tiles = [
                        (ps_a if t < 2 else ps_z).tile(
                            [P, D],
                            F32,
                            tag=("a" if t < 2 else "z"),
                            name=f"vps_{base}_{t}",
                        )
                        for t in range(4)
                    ]
                    for k in range(KO):
                        for t in range(4):
                            tci = base + t
                            for hh in range(2):
                                nc.tensor.matmul(
                                    tiles[t][:, hh * 512 : (hh + 1) * 512],
                                    Xt[:, k, tci * P : (tci + 1) * P],
                                    Wv[:, k, hh * 512 : (hh + 1) * 512],
                                    start=(k == 0),
                                    stop=(k == KO - 1),
                                )
                    for t in range(4):
                        tci = base + t
                        nc.vector.tensor_tensor(
                            Vg[:, tci],
                            tiles[t].rearrange("p (h d) -> p h d", d=DK),
                            bv_r.rearrange("p (h d) -> p h d", d=DK),
                            ADD,
                        )

            # ---- phase B: per-head Q/K projection + attention ----
            with (
                tc.tile_pool(name="qk", bufs=1) as qk,
                tc.tile_pool(name="pt_pool", bufs=2) as pt_pool,
                tc.tile_pool(name="s2_pool", bufs=1) as s2_pool,
                tc.tile_pool(name="ab_pool", bufs=2) as ab_pool,
                tc.tile_pool(name="r_pool", bufs=2) as r_pool,
            ):

                def proj_head(h, w_sb, b_p, tagname):
                    dst = qk.tile([P, T], BF, tag=tagname)
                    for t2 in range(2):
                        ps2 = ps_a.tile([P, D], F32, tag="a", name=f"qk_{h}_{t2}")
                        for k in range(KO):
                            for hf in range(2):
                                nc.tensor.matmul(
                                    ps2[:, hf * 512 : (hf + 1) * 512],
                                    w_sb[:, k, h * P : (h + 1) * P],
                                    Xt[:, k, t2 * N + hf * 512 : t2 * N + (hf + 1) * 512],
                                    start=(k == 0),
                                    stop=(k == KO - 1),
                                )
                        nc.vector.tensor_tensor(
                            dst[:, t2 * N : (t2 + 1) * N],
                            ps2[:],
                            b_p[:, h : h + 1].to_broadcast((P, N)),
                            ADD,
                        )
                    return dst

                def finish_prev(prev, zt_ps, tagn):
                    """Denominator matmul + reciprocal + normalized Zt copy."""
                    pAb, ph, pq0 = prev[1], prev[2], prev[4]
                    d_ps = ps_z.tile([P, N], F32, tag="z", name=f"d_{tagn}")
                    for nh in range(2):
                        nc.tensor.matmul(
                            d_ps[:, nh * 512 : (nh + 1) * 512],
                            ones128[:],
                            pAb[:, nh * 512 : (nh + 1) * 512],
                            start=True,
                            stop=True,
                        )
                    R = r_pool.tile([P, N], F32, tag="r")
                    nc.vector.reciprocal(R[:], d_ps[:])
                    nc.vector.tensor_tensor(
                        Zt[:, ph, pq0 : pq0 + N], zt_ps[:], R[:], MULT
                    )

                def unit_step(h, ch, Qth, Kth, prev):
                    """Scores+exp for unit (h, ch); attn@V + normalize for prev."""
                    chp = (1 - ch) if h < N_CROSS else ch  # kv channel
                    q0 = ch * N
                    m0 = chp * N
                    PT = pt_pool.tile([P, KO, N], BF, tag="pt")
                    S2 = s2_pool.tile([P, 4, N], BF, tag="s2")
                    zt_ps = None
                    if prev is not None:
                        pPT, pchp, ph = prev[0], prev[3], prev[2]
                        zt_ps = ps_z.tile([P, N], F32, tag="z", name=f"zt_{h}_{ch}")
                    for mi in range(KO):
                        ps2 = ps_a.tile([P, N], F32, tag="a", name=f"s_{h}_{ch}_{mi}")
                        for nh in range(2):
                            nc.tensor.matmul(
                                ps2[:, nh * 512 : (nh + 1) * 512],
                                Kth[:, m0 + mi * P : m0 + (mi + 1) * P],
                                Qth[:, q0 + nh * 512 : q0 + (nh + 1) * 512],
                                start=True,
                                stop=True,
                            )
                        nc.scalar.activation(PT[:, mi], ps2[:], EXP, scale=SCALE)
                        if mi % 2 == 1:
                            nc.vector.tensor_tensor(
                                S2[:, mi // 2], PT[:, mi - 1], PT[:, mi], ADD
                            )
                        if prev is not None:
                            vch = pchp * KO + mi
                            for nh in range(2):
                                nc.tensor.matmul(
                                    zt_ps[:, nh * 512 : (nh + 1) * 512],
                                    Vg[:, vch, ph, :],
                                    pPT[:, mi, nh * 512 : (nh + 1) * 512],
                                    start=(mi == 0),
                                    stop=(mi == KO - 1),
                                )
                    if prev is not None:
                        finish_prev(prev, zt_ps, f"{h}_{ch}")
                    nc.vector.tensor_tensor(S2[:, 0], S2[:, 0], S2[:, 1], ADD)
                    nc.vector.tensor_tensor(S2[:, 2], S2[:, 2], S2[:, 3], ADD)
                    Ab = ab_pool.tile([P, N], BF, tag="ab")
                    nc.vector.tensor_tensor(Ab[:], S2[:, 0], S2[:, 2], ADD)
                    return (PT, Ab, h, chp, q0)

                def attnv_flush(prev):
                    pPT, pchp, ph = prev[0], prev[3], prev[2]
                    zt_ps = ps_z.tile([P, N], F32, tag="z", name="zt_flush")
                    for mi in range(KO):
                        vch = pchp * KO + mi
                        for nh in range(2):
                            nc.tensor.matmul(
                                zt_ps[:, nh * 512 : (nh + 1) * 512],
                                Vg[:, vch, ph, :],
                                pPT[:, mi, nh * 512 : (nh + 1) * 512],
                                start=(mi == 0),
                                stop=(mi == KO - 1),
                            )
                    finish_prev(prev, zt_ps, "flush")

                prev = None
                for h in range(H):
                    Qth = proj_head(h, Wq, bq_p, "qth")
                    Kth = proj_head(h, Wk, bk_p, "kth")
                    for ch in range(2):
                        prev = unit_step(h, ch, Qth, Kth, prev)
                attnv_flush(prev)

            # ---- phase C: output projection ----
            with (
                tc.tile_pool(name="wo_pool", bufs=1) as wo_pool,
                tc.tile_pool(name="y_pool", bufs=2) as y_pool,
            ):
                Wo = wo_pool.tile([P, KO, D], BF)
                nc.gpsimd.dma_start(Wo[:, 0, 0:512], wo_r[:, 0, 0:512])
                nc.gpsimd.dma_start(Wo[:, 0, 512:1024], wo_r[:, 0, 512:1024])
                for k in range(1, KO):
                    nc.gpsimd.dma_start(Wo[:, k, :], wo_r[:, k, :])
                for tci in range(TC):
                    pool, tag = (ps_a, "a") if tci % 2 == 0 else (ps_z, "z")
                    ps2 = pool.tile([P, D], F32, tag=tag, name=f"o_{tci}")
                    for k in range(KO):
                        for hf in range(2):
                            nc.tensor.matmul(
                                ps2[:, hf * 512 : (hf + 1) * 512],
                                Zt[:, k, tci * P : (tci + 1) * P],
                                Wo[:, k, hf * 512 : (hf + 1) * 512],
                                start=(k == 0),
                                stop=(k == KO - 1),
                            )
                    y = y_pool.tile([P, D], BF, tag="y")
                    nc.vector.tensor_tensor(y[:], ps2[:], bo_r[:], ADD)
                    nc.sync.dma_start(out_d[tci * P : (tci + 1) * P, :], y[:])
    return nc


def _get_program():
    if "nc" not in _CACHE:
        _legalize_install()
        _CACHE["nc"] = _build()
    return _CACHE["nc"]


def make_in_maps(inputs):
    x = np.asarray(inputs["x"], dtype=np.float32)
    bs2 = x.shape[0]
    n_cores = bs2 // 2
    bf = ml_dtypes.bfloat16

    weights = {
        name: np.ascontiguousarray(np.asarray(inputs[name], dtype=np.float32)).astype(
            bf
        )
        for name in ("Wq", "Wk", "Wv", "Wo")
    }
    biases = {
        name: np.ascontiguousarray(np.asarray(inputs[name], dtype=np.float32))
        for name in ("bq", "bk", "bv", "bo")
    }

    in_maps = []
    for c in range(n_cores):
        xt = np.ascontiguousarray(x[2 * c : 2 * c + 2].reshape(T, D).T).astype(bf)
        in_maps.append(
            {
                "xt": xt,
                "wq": weights["Wq"],
                "wk": weights["Wk"],
                "wv": weights["Wv"],
                "wo": weights["Wo"],
                "bq": biases["bq"],
                "bk": biases["bk"],
                "bv": biases["bv"],
                "bo": biases["bo"],
            }
        )
    return in_maps


def kernel(**inputs):
    bs2 = np.asarray(inputs["x"]).shape[0]
    n_cores = bs2 // 2
    in_maps = make_in_maps(inputs)
    nc = _get_program()
    res = run_bass_kernel_spmd(nc, in_maps, core_ids=list(range(n_cores)))
    out = np.empty((bs2, N, D), dtype=np.float32)
    for c in range(n_cores):
        out[2 * c : 2 * c + 2] = (
            res.results[c]["out"].astype(np.float32).reshape(2, N, D)
        )
    return out


# revision 17
# speedup vs baseline: 1.2479x; 1.2479x over previous
"""Cross-channel multi-head attention on 8 Trainium2 NeuronCores.

Sharding: data-parallel over the batch axis. bs2=16 sequences form bs=8
(batch, 2-channel) pairs; each core handles one pair fully locally
(cross-channel attention couples only the two channels of the same batch
element), so no collectives are needed.

Per core (T=2048 tokens = 2 channels x 1024 patches, D=1024, H=8 heads,
dk=128; heads 0..5 attend to the other channel's K/V, heads 6..7 to the
same channel):
  1. V = x @ Wv + bv in natural [T, D] layout (phase A), Qt/Kt = per-head
     [dk, T] projections (phase B) -- all matmul streams 512 wide so the
     PE's per-matmul LDWEIGHTS hides under the previous multiply.
  2. Per (head, channel) unit: S^T chunks = Kt-chunk^T x Qt (psum
     [128, 1024] spanning 2 banks, halves as separate matmul groups);
     P^T = exp(S^T/sqrt(dk)) via one [128,1024] scalar ACTIVATE per chunk.
  3. attn@V with V as the STATIONARY operand and P^T streaming 512 wide:
     Z^T[dk, n] accumulates directly in psum -- no PE transposes, no
     ones-column. Softmax denominators: DVE pair-add tree over the 8 P^T
     chunks (bf16), then an all-ones [128,128] stationary matmul whose
     output is the partition sum REPLICATED across all 128 partitions
     (f32 accumulate), DVE reciprocal psum->sbuf; the normalize then
     multiplies along the free dim in the psum->Zt copy.
  4. out = Zt-chunks^T @ Wo + bo, stored bf16 (host upcasts to f32).

All matmuls bf16 with f32 PSUM accumulation. Denominator tree in bf16
(values ~1e2..1e4, well within range; adds ~0.3% rel err, total ~0.8%
vs the 2e-2 gate). The host pre-transposes/casts x to bf16 [D, T] per
core and casts the weights to bf16.
"""

import sys

if "/opt/trn_rl_repo" not in sys.path:
    sys.path.insert(0, "/opt/trn_rl_repo")

import numpy as np
import ml_dtypes

import concourse.bass as bass
import concourse.bass_isa as bass_isa
import concourse.tile as tile
from concourse import mybir
from concourse.bass_utils import run_bass_kernel_spmd

# Walrus in this container rejects >1 wait condition on TPB_CTRL ops
# (Tile's kernel-tail drain carries one per active proc). Split them.
import os

_here = os.path.dirname(os.path.abspath(__file__))
if _here not in sys.path:
    sys.path.insert(0, _here)
try:
    import bir_legalize
except ImportError:  # graded in a bare dir: fall back to inline copy
    bir_legalize = None

N = 1024  # patches per channel
D = 1024
H = 8
DK = 128
N_CROSS = 6
T = 2 * N  # tokens per core (2 channels of one batch element)
P = 128
KO = D // P  # 8 outer chunks of the 1024-wide dims
TC = T // P  # 16 token chunks
BF = mybir.dt.bfloat16
F32 = mybir.dt.float32
SCALE = 1.0 / float(np.sqrt(DK))
EXP = mybir.ActivationFunctionType.Exp
ADD = mybir.AluOpType.add
MULT = mybir.AluOpType.mult

_CACHE = {}


def _legalize_install():
    if bir_legalize is not None:
        bir_legalize.install()
        return
    # Inline fallback (kernel.py must be self-contained when graded).
    import json
    import concourse.bass2jax as bass2jax
    from concourse.bass_utils import compile_bir_kernel as _orig

    if getattr(bass2jax.compile_bir_kernel, "_legalized", False):
        return

    OPCODE_MAX = {}
    SKIP = set()

    def _legalize(bir_json):
        d = json.loads(bir_json)
        changed = False
        for fn in d.get("functions", []):
            for bb in fn.get("blocks") or fn.get("basicblocks") or []:
                out = []
                for inst in bb.get("instructions", []):
                    sync = inst.get("sync_info") or {}
                    waits = sync.get("on_wait") or []
                    cap = OPCODE_MAX.get(inst.get("opcode"), 1)
                    if len(waits) > cap and inst.get("opcode") not in SKIP:
                        extra, keep = waits[:-cap], waits[-cap:]
                        for i, w in enumerate(extra):
                            out.append(
                                {
                                    "debug": inst.get("debug", 0),
                                    "engine": inst["engine"],
                                    "ins": [],
                                    "outs": [],
                                    "is_reset_sema": False,
                                    "name": f"{inst['name']}-sw{i}",
                                    "opcode": "Drain",
                                    "sync_info": {"on_update": [], "on_wait": [w]},
                                }
                            )
                        sync["on_wait"] = keep
                        inst["sync_info"] = sync
                        changed = True
                    out.append(inst)
                bb["instructions"] = out
        return json.dumps(d).encode() if changed else bir_json

    def compile_bir_kernel(bir_json, tmpdir, neff_name="file.neff"):
        return _orig(_legalize(bir_json), tmpdir, neff_name)

    compile_bir_kernel._legalized = True
    bass2jax.compile_bir_kernel = compile_bir_kernel


def _bcast_rows(ap, p):
    """Replicate a 1-D DRAM AP across p partitions (stride-0 partition dim)."""
    return bass.AP(tensor=ap.tensor, offset=ap.offset, ap=[[0, p], *ap.ap])


def _build():
    nc = bass.Bass()

    xt_d = nc.dram_tensor("xt", [D, T], BF, kind="ExternalInput").ap()
    wq_d = nc.dram_tensor("wq", [D, D], BF, kind="ExternalInput").ap()
    wk_d = nc.dram_tensor("wk", [D, D], BF, kind="ExternalInput").ap()
    wv_d = nc.dram_tensor("wv", [D, D], BF, kind="ExternalInput").ap()
    wo_d = nc.dram_tensor("wo", [D, D], BF, kind="ExternalInput").ap()
    bq_d = nc.dram_tensor("bq", [D], F32, kind="ExternalInput").ap()
    bk_d = nc.dram_tensor("bk", [D], F32, kind="ExternalInput").ap()
    bv_d = nc.dram_tensor("bv", [D], F32, kind="ExternalInput").ap()
    bo_d = nc.dram_tensor("bo", [D], F32, kind="ExternalInput").ap()
    out_d = nc.dram_tensor("out", [T, D], BF, kind="ExternalOutput").ap()

    xt_r = xt_d.rearrange("(o p) t -> p o t", p=P)
    wq_r = wq_d.rearrange("(o p) f -> p o f", p=P)
    wk_r = wk_d.rearrange("(o p) f -> p o f", p=P)
    wv_r = wv_d.rearrange("(o p) f -> p o f", p=P)
    wo_r = wo_d.rearrange("(o p) f -> p o f", p=P)

    with tile.TileContext(nc) as tc:
        with (
            tc.tile_pool(name="consts", bufs=1) as consts,
            tc.tile_pool(name="big", bufs=1) as big,
            tc.tile_pool(name="xt_w", bufs=1) as xt_w,
            tc.tile_pool(name="ps_a", bufs=2, space="PSUM") as ps_a,
            tc.tile_pool(name="ps_z", bufs=2, space="PSUM") as ps_z,
        ):
            Vg = big.tile([P, TC, H, DK], BF)  # V natural, per token-chunk/head
            Zt = big.tile([P, H, T], BF)  # attention out, [dout, T]
            bq_p = consts.tile([P, KO], F32)
            bk_p = consts.tile([P, KO], F32)
            bo_r = consts.tile([P, D], F32)

            # ---- phase A: V projection ----
            with tc.tile_pool(name="wv_pool", bufs=1) as wv_pool:
                Xt = xt_w.tile([P, KO, T], BF)
                Wq = xt_w.tile([P, KO, D], BF)
                Wk = xt_w.tile([P, KO, D], BF)
                Wv = wv_pool.tile([P, KO, D], BF)
                bv_r = wv_pool.tile([P, D], F32)

                # DMA issue order is the startup critical path. sync carries
                # the first-half Xt k-chunks (consumed k-ascending by batch
                # 0/1), scalar carries Wv then biases then Wq/Wk, gpsimd
                # carries the second-half Xt (batches 2/3, prefetched).
                nc.sync.dma_start(Xt[:, 0, 0:512], xt_r[:, 0, 0:512])
                nc.scalar.dma_start(Wv[:, 0, :], wv_r[:, 0, :])
                for k in range(1, KO):
                    nc.sync.dma_start(Xt[:, k, 0:512], xt_r[:, k, 0:512])
                    nc.scalar.dma_start(Wv[:, k, :], wv_r[:, k, :])
                for k in range(KO):
                    nc.sync.dma_start(Xt[:, k, 512:1024], xt_r[:, k, 512:1024])
                    nc.gpsimd.dma_start(Xt[:, k, 1024:2048], xt_r[:, k, 1024:2048])
                nc.scalar.dma_start(bq_p[:], bq_d.rearrange("(o p) -> p o", p=P))
                nc.scalar.dma_start(bk_p[:], bk_d.rearrange("(o p) -> p o", p=P))
                nc.scalar.dma_start(bv_r[:], _bcast_rows(bv_d, P))
                for k in range(KO):
                    nc.scalar.dma_start(Wq[:, k, :], wq_r[:, k, :])
                    nc.scalar.dma_start(Wk[:, k, :], wk_r[:, k, :])
                nc.scalar.dma_start(bo_r[:], _bcast_rows(bo_d, P))

                # Warm the PE HAM clock gate with throwaway accumulating
                # matmuls so the first real matmuls run at 2.4 GHz; sized to
                # roughly cover the first-input DMA window.
                warm_in = consts.tile([P, P], BF)
                nc.vector.memset(warm_in[:], 0.0)
                warm_rhs = consts.tile([P, 512], BF)
                nc.vector.memset(warm_rhs[:], 0.0)
                ones128 = consts.tile([P, P], BF)
                nc.vector.memset(ones128[:], 1.0)
                for g in range(2):
                    wps = ps_a.tile([P, D], F32, tag="a", name=f"warm_{g}")
                    for k in range(8):
                        nc.tensor.matmul(
                            wps[:, 0:512],
                            warm_in[:],
                            warm_rhs[:],
                            start=(k == 0),
                            stop=(k == 7),
                        )

                # V natural = Xt-chunk.T @ Wv, 4 two-bank psum groups in
                # flight (8 banks) so the PE has backlog while inputs stream.
                for base in range(0, TC, 4):
                    tiles = [
                        (ps_a if t < 2 else ps_z).tile(
                            [P, D],
                            F32,
                            tag=("a" if t < 2 else "z"),
                            name=f"vps_{base}_{t}",
                        )
                        for t in range(4)
                    ]
                    for k in range(KO):
                        for t in range(4):
                            tci = base + t
                            for hh in range(2):
                                nc.tensor.matmul(
                                    tiles[t][:, hh * 512 : (hh + 1) * 512],
                                    Xt[:, k, tci * P : (tci + 1) * P],
                                    Wv[:, k, hh * 512 : (hh + 1) * 512],
                                    start=(k == 0),
                                    stop=(k == KO - 1),
                                )
                    for t in range(4):
                        tci = base + t
                        nc.vector.tensor_tensor(
                            Vg[:, tci],
                            tiles[t].rearrange("p (h d) -> p h d", d=DK),
                            bv_r.rearrange("p (h d) -> p h d", d=DK),
                            ADD,
                        )

            # ---- phase B: per-head Q/K projection + attention ----
            with (
                tc.tile_pool(name="qk", bufs=2) as qk,
                tc.tile_pool(name="pt_pool", bufs=2) as pt_pool,
                tc.tile_pool(name="s2_pool", bufs=1) as s2_pool,
                tc.tile_pool(name="ab_pool", bufs=2) as ab_pool,
                tc.tile_pool(name="r_pool", bufs=2) as r_pool,
            ):

                def proj_head(h, w_sb, b_p, tagname):
                    dst = qk.tile([P, T], BF, tag=tagname)
                    for t2 in range(2):
                        ps2 = ps_z.tile([P, D], F32, tag="z", name=f"qk_{h}_{t2}")
                        for k in range(KO):
                            for hf in range(2):
                                nc.tensor.matmul(
                                    ps2[:, hf * 512 : (hf + 1) * 512],
                                    w_sb[:, k, h * P : (h + 1) * P],
                                    Xt[:, k, t2 * N + hf * 512 : t2 * N + (hf + 1) * 512],
                                    start=(k == 0),
                                    stop=(k == KO - 1),
                                )
                        nc.vector.tensor_tensor(
                            dst[:, t2 * N : (t2 + 1) * N],
                            ps2[:],
                            b_p[:, h : h + 1].to_broadcast((P, N)),
                            ADD,
                        )
                    return dst

                def _scalar_recip(out, in_):
                    # The scalar-engine Reciprocal table is plenty accurate
                    # for the softmax denominator (~1e-5 rel) and runs at
                    # ~1.4us per [128,1024] vs 6.5us on the DVE. bass's
                    # activation() refuses func=Reciprocal, so emit directly.
                    ins = [nc.scalar.lower_ap(in_)]
                    for arg in (0.0, 1.0, 0.0):
                        ins.append(mybir.ImmediateValue(dtype=F32, value=arg))
                    nc.scalar.add_instruction(
                        mybir.InstActivation(
                            name=nc.get_next_instruction_name(),
                            func=mybir.ActivationFunctionType.Reciprocal,
                            ins=ins,
                            outs=[nc.scalar.lower_ap(out)],
                        )
                    )

                def start_prev(prev, tagn):
                    """Denominator matmul + scalar reciprocal, emitted at the
                    top of the next window so the chain completes before the
                    normalize-copy at the window's end."""
                    pAb = prev[1]
                    d_ps = ps_z.tile([P, N], F32, tag="z", name=f"d_{tagn}")
                    for nh in range(2):
                        nc.tensor.matmul(
                            d_ps[:, nh * 512 : (nh + 1) * 512],
                            ones128[:],
                            pAb[:, nh * 512 : (nh + 1) * 512],
                            start=True,
                            stop=True,
                        )
                    R = r_pool.tile([P, N], F32, tag="r")
                    _scalar_recip(R[:], d_ps[:])
                    return R

                def unit_step(h, ch, Qth, Kth, prev):
                    """Scores+exp for unit (h, ch); attn@V + normalize for prev."""
                    chp = (1 - ch) if h < N_CROSS else ch  # kv channel
                    q0 = ch * N
                    m0 = chp * N
                    PT = pt_pool.tile([P, KO, N], BF, tag="pt")
                    S2 = s2_pool.tile([P, 4, N], BF, tag="s2")
                    zt_ps = None
                    if prev is not None:
                        pPT, pchp, ph = prev[0], prev[3], prev[2]
                        R = start_prev(prev, f"{h}_{ch}")
                        zt_ps = ps_z.tile([P, N], F32, tag="z", name=f"zt_{h}_{ch}")
                    for mi in range(KO):
                        ps2 = ps_a.tile([P, N], F32, tag="a", name=f"s_{h}_{ch}_{mi}")
                        for nh in range(2):
                            nc.tensor.matmul(
                                ps2[:, nh * 512 : (nh + 1) * 512],
                                Kth[:, m0 + mi * P : m0 + (mi + 1) * P],
                                Qth[:, q0 + nh * 512 : q0 + (nh + 1) * 512],
                                start=True,
                                stop=True,
                            )
                        nc.scalar.activation(PT[:, mi], ps2[:], EXP, scale=SCALE)
                        if mi % 2 == 1:
                            nc.vector.tensor_tensor(
                                S2[:, mi // 2], PT[:, mi - 1], PT[:, mi], ADD
                            )
                        if prev is not None:
                            vch = pchp * KO + mi
                            for nh in range(2):
                                nc.tensor.matmul(
                                    zt_ps[:, nh * 512 : (nh + 1) * 512],
                                    Vg[:, vch, ph, :],
                                    pPT[:, mi, nh * 512 : (nh + 1) * 512],
                                    start=(mi == 0),
                                    stop=(mi == KO - 1),
                                )
                    if prev is not None:
                        nc.vector.tensor_tensor(
                            Zt[:, prev[2], prev[4] : prev[4] + N],
                            zt_ps[:],
                            R[:],
                            MULT,
                        )
                    nc.vector.tensor_tensor(S2[:, 0], S2[:, 0], S2[:, 1], ADD)
                    nc.vector.tensor_tensor(S2[:, 2], S2[:, 2], S2[:, 3], ADD)
                    Ab = ab_pool.tile([P, N], BF, tag="ab")
                    nc.vector.tensor_tensor(Ab[:], S2[:, 0], S2[:, 2], ADD)
                    return (PT, Ab, h, chp, q0)

                def attnv_flush(prev):
                    pPT, pchp, ph = prev[0], prev[3], prev[2]
                    R = start_prev(prev, "flush")
                    zt_ps = ps_z.tile([P, N], F32, tag="z", name="zt_flush")
                    for mi in range(KO):
                        vch = pchp * KO + mi
                        for nh in range(2):
                            nc.tensor.matmul(
                                zt_ps[:, nh * 512 : (nh + 1) * 512],
                                Vg[:, vch, ph, :],
                                pPT[:, mi, nh * 512 : (nh + 1) * 512],
                                start=(mi == 0),
                                stop=(mi == KO - 1),
                            )
                    nc.vector.tensor_tensor(
                        Zt[:, ph, prev[4] : prev[4] + N], zt_ps[:], R[:], MULT
                    )

                # Q/K are projected one head AHEAD, interleaved between the
                # attention unit loops, so the scalar exp backlog and the
                # denominator chain drain during exp-independent PE work.
                Qcur = proj_head(0, Wq, bq_p, "qth")
                Kcur = proj_head(0, Wk, bk_p, "kth")
                prev = None
                Qnext = Knext = None
                for h in range(H):
                    prev = unit_step(h, 0, Qcur, Kcur, prev)
                    if h < H - 1:
                        Qnext = proj_head(h + 1, Wq, bq_p, "qth")
                    prev = unit_step(h, 1, Qcur, Kcur, prev)
                    if h < H - 1:
                        Knext = proj_head(h + 1, Wk, bk_p, "kth")
                        Qcur, Kcur = Qnext, Knext
                attnv_flush(prev)

            # ---- phase C: output projection ----
            with (
                tc.tile_pool(name="wo_pool", bufs=1) as wo_pool,
                tc.tile_pool(name="y_pool", bufs=2) as y_pool,
            ):
                Wo = wo_pool.tile([P, KO, D], BF)
                nc.gpsimd.dma_start(Wo[:, 0, 0:512], wo_r[:, 0, 0:512])
                nc.gpsimd.dma_start(Wo[:, 0, 512:1024], wo_r[:, 0, 512:1024])
                for k in range(1, KO):
                    nc.gpsimd.dma_start(Wo[:, k, :], wo_r[:, k, :])
                for tci in range(TC):
                    pool, tag = (ps_a, "a") if tci % 2 == 0 else (ps_z, "z")
                    ps2 = pool.tile([P, D], F32, tag=tag, name=f"o_{tci}")
                    for k in range(KO):
                        for hf in range(2):
                            nc.tensor.matmul(
                                ps2[:, hf * 512 : (hf + 1) * 512],
                                Zt[:, k, tci * P : (tci + 1) * P],
                                Wo[:, k, hf * 512 : (hf + 1) * 512],
                                start=(k == 0),
                                stop=(k == KO - 1),
                            )
                    y = y_pool.tile([P, D], BF, tag="y")
                    nc.vector.tensor_tensor(y[:], ps2[:], bo_r[:], ADD)
                    nc.sync.dma_start(out_d[tci * P : (tci + 1) * P, :], y[:])
    return nc


def _get_program():
    if "nc" not in _CACHE:
        _legalize_install()
        _CACHE["nc"] = _build()
    return _CACHE["nc"]


def make_in_maps(inputs):
    x = np.asarray(inputs["x"], dtype=np.float32)
    bs2 = x.shape[0]
    n_cores = bs2 // 2
    bf = ml_dtypes.bfloat16

    weights = {
        name: np.ascontiguousarray(np.asarray(inputs[name], dtype=np.float32)).astype(
            bf
        )
        for name in ("Wq", "Wk", "Wv", "Wo")
    }
    biases = {
        name: np.ascontiguousarray(np.asarray(inputs[name], dtype=np.float32))
        for name in ("bq", "bk", "bv", "bo")
    }

    in_maps = []
    for c in range(n_cores):
        xt = np.ascontiguousarray(x[2 * c : 2 * c + 2].reshape(T, D).T).astype(bf)
        in_maps.append(
            {
                "xt": xt,
                "wq": weights["Wq"],
                "wk": weights["Wk"],
                "wv": weights["Wv"],
                "wo": weights["Wo"],
                "bq": biases["bq"],
                "bk": biases["bk"],
                "bv": biases["bv"],
                "bo": biases["bo"],
            }
        )
    return in_maps


def kernel(**inputs):
    bs2 = np.asarray(inputs["x"]).shape[0]
    n_cores = bs2 // 2
    in_maps = make_in_maps(inputs)
    nc = _get_program()
    res = run_bass_kernel_spmd(nc, in_maps, core_ids=list(range(n_cores)))
    out = np.empty((bs2, N, D), dtype=np.float32)
    for c in range(n_cores):
        out[2 * c : 2 * c + 2] = (
            res.results[c]["out"].astype(np.float32).reshape(2, N, D)
        )
    return out


# revision 20
# speedup vs baseline: 1.2536x; 1.0046x over previous
"""Cross-channel multi-head attention on 8 Trainium2 NeuronCores.

Sharding: data-parallel over the batch axis. bs2=16 sequences form bs=8
(batch, 2-channel) pairs; each core handles one pair fully locally
(cross-channel attention couples only the two channels of the same batch
element), so no collectives are needed.

Per core (T=2048 tokens = 2 channels x 1024 patches, D=1024, H=8 heads,
dk=128; heads 0..5 attend to the other channel's K/V, heads 6..7 to the
same channel):
  1. V = x @ Wv + bv in natural [T, D] layout (phase A), Qt/Kt = per-head
     [dk, T] projections (phase B) -- all matmul streams 512 wide so the
     PE's per-matmul LDWEIGHTS hides under the previous multiply.
  2. Per (head, channel) unit: S^T chunks = Kt-chunk^T x Qt (psum
     [128, 1024] spanning 2 banks, halves as separate matmul groups);
     P^T = exp(S^T/sqrt(dk)) via one [128,1024] scalar ACTIVATE per chunk.
  3. attn@V with V as the STATIONARY operand and P^T streaming 512 wide:
     Z^T[dk, n] accumulates directly in psum -- no PE transposes, no
     ones-column. Softmax denominators: DVE pair-add tree over the 8 P^T
     chunks (bf16), then an all-ones [128,128] stationary matmul whose
     output is the partition sum REPLICATED across all 128 partitions
     (f32 accumulate), DVE reciprocal psum->sbuf; the normalize then
     multiplies along the free dim in the psum->Zt copy.
  4. out = Zt-chunks^T @ Wo + bo, stored bf16 (host upcasts to f32).

All matmuls bf16 with f32 PSUM accumulation. Denominator tree in bf16
(values ~1e2..1e4, well within range; adds ~0.3% rel err, total ~0.8%
vs the 2e-2 gate). The host pre-transposes/casts x to bf16 [D, T] per
core and casts the weights to bf16.
"""

import sys

if "/opt/trn_rl_repo" not in sys.path:
    sys.path.insert(0, "/opt/trn_rl_repo")

import numpy as np
import ml_dtypes

import concourse.bass as bass
import concourse.bass_isa as bass_isa
import concourse.tile as tile
from concourse import mybir
from concourse.bass_utils import run_bass_kernel_spmd

# Walrus in this container rejects >1 wait condition on TPB_CTRL ops
# (Tile's kernel-tail drain carries one per active proc). Split them.
import os

_here = os.path.dirname(os.path.abspath(__file__))
if _here not in sys.path:
    sys.path.insert(0, _here)
try:
    import bir_legalize
except ImportError:  # graded in a bare dir: fall back to inline copy
    bir_legalize = None

N = 1024  # patches per channel
D = 1024
H = 8
DK = 128
N_CROSS = 6
T = 2 * N  # tokens per core (2 channels of one batch element)
P = 128
KO = D // P  # 8 outer chunks of the 1024-wide dims
TC = T // P  # 16 token chunks
BF = mybir.dt.bfloat16
F32 = mybir.dt.float32
SCALE = 1.0 / float(np.sqrt(DK))
EXP = mybir.ActivationFunctionType.Exp
ADD = mybir.AluOpType.add
MULT = mybir.AluOpType.mult

_CACHE = {}


def _legalize_install():
    if bir_legalize is not None:
        bir_legalize.install()
        return
    # Inline fallback (kernel.py must be self-contained when graded).
    import json
    import concourse.bass2jax as bass2jax
    from concourse.bass_utils import compile_bir_kernel as _orig

    if getattr(bass2jax.compile_bir_kernel, "_legalized", False):
        return

    OPCODE_MAX = {}
    SKIP = set()

    def _legalize(bir_json):
        d = json.loads(bir_json)
        changed = False
        for fn in d.get("functions", []):
            for bb in fn.get("blocks") or fn.get("basicblocks") or []:
                out = []
                for inst in bb.get("instructions", []):
                    sync = inst.get("sync_info") or {}
                    waits = sync.get("on_wait") or []
                    cap = OPCODE_MAX.get(inst.get("opcode"), 1)
                    if len(waits) > cap and inst.get("opcode") not in SKIP:
                        extra, keep = waits[:-cap], waits[-cap:]
                        for i, w in enumerate(extra):
                            out.append(
                                {
                                    "debug": inst.get("debug", 0),
                                    "engine": inst["engine"],
                                    "ins": [],
                                    "outs": [],
                                    "is_reset_sema": False,
                                    "name": f"{inst['name']}-sw{i}",
                                    "opcode": "Drain",
                                    "sync_info": {"on_update": [], "on_wait": [w]},
                                }
                            )
                        sync["on_wait"] = keep
                        inst["sync_info"] = sync
                        changed = True
                    out.append(inst)
                bb["instructions"] = out
        return json.dumps(d).encode() if changed else bir_json

    def compile_bir_kernel(bir_json, tmpdir, neff_name="file.neff"):
        return _orig(_legalize(bir_json), tmpdir, neff_name)

    compile_bir_kernel._legalized = True
    bass2jax.compile_bir_kernel = compile_bir_kernel


def _bcast_rows(ap, p):
    """Replicate a 1-D DRAM AP across p partitions (stride-0 partition dim)."""
    return bass.AP(tensor=ap.tensor, offset=ap.offset, ap=[[0, p], *ap.ap])


def _build():
    nc = bass.Bass()

    xt_d = nc.dram_tensor("xt", [D, T], BF, kind="ExternalInput").ap()
    wq_d = nc.dram_tensor("wq", [D, D], BF, kind="ExternalInput").ap()
    wk_d = nc.dram_tensor("wk", [D, D], BF, kind="ExternalInput").ap()
    wv_d = nc.dram_tensor("wv", [D, D], BF, kind="ExternalInput").ap()
    wo_d = nc.dram_tensor("wo", [D, D], BF, kind="ExternalInput").ap()
    bq_d = nc.dram_tensor("bq", [D], F32, kind="ExternalInput").ap()
    bk_d = nc.dram_tensor("bk", [D], F32, kind="ExternalInput").ap()
    bv_d = nc.dram_tensor("bv", [D], F32, kind="ExternalInput").ap()
    bo_d = nc.dram_tensor("bo", [D], F32, kind="ExternalInput").ap()
    out_d = nc.dram_tensor("out", [T, D], BF, kind="ExternalOutput").ap()

    xt_r = xt_d.rearrange("(o p) t -> p o t", p=P)
    wq_r = wq_d.rearrange("(o p) f -> p o f", p=P)
    wk_r = wk_d.rearrange("(o p) f -> p o f", p=P)
    wv_r = wv_d.rearrange("(o p) f -> p o f", p=P)
    wo_r = wo_d.rearrange("(o p) f -> p o f", p=P)

    with tile.TileContext(nc) as tc:
        with (
            tc.tile_pool(name="consts", bufs=1) as consts,
            tc.tile_pool(name="big", bufs=1) as big,
            tc.tile_pool(name="xt_w", bufs=1) as xt_w,
            tc.tile_pool(name="ps_a", bufs=2, space="PSUM") as ps_a,
            tc.tile_pool(name="ps_z", bufs=2, space="PSUM") as ps_z,
        ):
            Vg = big.tile([P, TC, H, DK], BF)  # V natural, per token-chunk/head
            Zt = big.tile([P, H, T], BF)  # attention out, [dout, T]
            bq_p = consts.tile([P, KO], F32)
            bk_p = consts.tile([P, KO], F32)
            bo_r = consts.tile([P, D], F32)

            # ---- phase A: V projection ----
            with tc.tile_pool(name="wv_pool", bufs=1) as wv_pool:
                Xt = xt_w.tile([P, KO, T], BF)
                Wq = xt_w.tile([P, KO, D], BF)
                Wk = xt_w.tile([P, KO, D], BF)
                Wv = wv_pool.tile([P, KO, D], BF)
                bv_r = wv_pool.tile([P, D], F32)

                # DMA issue order is the startup critical path. sync carries
                # the first-half Xt k-chunks (consumed k-ascending by batch
                # 0/1), scalar carries Wv then biases then Wq/Wk, gpsimd
                # carries the second-half Xt (batches 2/3, prefetched).
                nc.sync.dma_start(Xt[:, 0, 0:512], xt_r[:, 0, 0:512])
                nc.scalar.dma_start(Wv[:, 0, :], wv_r[:, 0, :])
                for k in range(1, KO):
                    nc.sync.dma_start(Xt[:, k, 0:512], xt_r[:, k, 0:512])
                    nc.scalar.dma_start(Wv[:, k, :], wv_r[:, k, :])
                for k in range(KO):
                    nc.sync.dma_start(Xt[:, k, 512:1024], xt_r[:, k, 512:1024])
                    nc.gpsimd.dma_start(Xt[:, k, 1024:2048], xt_r[:, k, 1024:2048])
                nc.scalar.dma_start(bq_p[:], bq_d.rearrange("(o p) -> p o", p=P))
                nc.scalar.dma_start(bk_p[:], bk_d.rearrange("(o p) -> p o", p=P))
                nc.scalar.dma_start(bv_r[:], _bcast_rows(bv_d, P))
                for k in range(KO):
                    nc.scalar.dma_start(Wq[:, k, :], wq_r[:, k, :])
                    nc.scalar.dma_start(Wk[:, k, :], wk_r[:, k, :])
                nc.scalar.dma_start(bo_r[:], _bcast_rows(bo_d, P))

                # Warm the PE HAM clock gate with throwaway accumulating
                # matmuls so the first real matmuls run at 2.4 GHz; sized to
                # roughly cover the first-input DMA window.
                warm_in = consts.tile([P, P], BF)
                nc.vector.memset(warm_in[:], 0.0)
                warm_rhs = consts.tile([P, 512], BF)
                nc.vector.memset(warm_rhs[:], 0.0)
                ones128 = consts.tile([P, P], BF)
                nc.vector.memset(ones128[:], 1.0)
                for g in range(2):
                    wps = ps_a.tile([P, D], F32, tag="a", name=f"warm_{g}")
                    for k in range(8):
                        nc.tensor.matmul(
                            wps[:, 0:512],
                            warm_in[:],
                            warm_rhs[:],
                            start=(k == 0),
                            stop=(k == 7),
                        )

                # V natural = Xt-chunk.T @ Wv, 4 two-bank psum groups in
                # flight (8 banks) so the PE has backlog while inputs stream.
                for base in range(0, TC, 4):
                    tiles = [
                        (ps_a if t < 2 else ps_z).tile(
                            [P, D],
                            F32,
                            tag=("a" if t < 2 else "z"),
                            name=f"vps_{base}_{t}",
                        )
                        for t in range(4)
                    ]
                    for k in range(KO):
                        for t in range(4):
                            tci = base + t
                            for hh in range(2):
                                nc.tensor.matmul(
                                    tiles[t][:, hh * 512 : (hh + 1) * 512],
                                    Xt[:, k, tci * P : (tci + 1) * P],
                                    Wv[:, k, hh * 512 : (hh + 1) * 512],
                                    start=(k == 0),
                                    stop=(k == KO - 1),
                                )
                    for t in range(4):
                        tci = base + t
                        nc.vector.tensor_tensor(
                            Vg[:, tci],
                            tiles[t].rearrange("p (h d) -> p h d", d=DK),
                            bv_r.rearrange("p (h d) -> p h d", d=DK),
                            ADD,
                        )

            # ---- phase B: per-head Q/K projection + attention ----
            with (
                tc.tile_pool(name="qk", bufs=2) as qk,
                tc.tile_pool(name="pt_pool", bufs=2) as pt_pool,
                tc.tile_pool(name="s2_pool", bufs=1) as s2_pool,
                tc.tile_pool(name="ab_pool", bufs=2) as ab_pool,
                tc.tile_pool(name="r_pool", bufs=2) as r_pool,
            ):

                def proj_head(h, w_sb, b_p, tagname):
                    dst = qk.tile([P, T], BF, tag=tagname)
                    for t2 in range(2):
                        ps2 = ps_z.tile([P, D], F32, tag="z", name=f"qk_{h}_{t2}")
                        for k in range(KO):
                            for hf in range(2):
                                nc.tensor.matmul(
                                    ps2[:, hf * 512 : (hf + 1) * 512],
                                    w_sb[:, k, h * P : (h + 1) * P],
                                    Xt[:, k, t2 * N + hf * 512 : t2 * N + (hf + 1) * 512],
                                    start=(k == 0),
                                    stop=(k == KO - 1),
                                )
                        nc.vector.tensor_tensor(
                            dst[:, t2 * N : (t2 + 1) * N],
                            ps2[:],
                            b_p[:, h : h + 1].to_broadcast((P, N)),
                            ADD,
                        )
                    return dst

                def _scalar_recip(out, in_):
                    # The scalar-engine Reciprocal table is plenty accurate
                    # for the softmax denominator (~1e-5 rel) and runs at
                    # ~1.4us per [128,1024] vs 6.5us on the DVE. bass's
                    # activation() refuses func=Reciprocal, so emit directly.
                    ins = [nc.scalar.lower_ap(in_)]
                    for arg in (0.0, 1.0, 0.0):
                        ins.append(mybir.ImmediateValue(dtype=F32, value=arg))
                    nc.scalar.add_instruction(
                        mybir.InstActivation(
                            name=nc.get_next_instruction_name(),
                            func=mybir.ActivationFunctionType.Reciprocal,
                            ins=ins,
                            outs=[nc.scalar.lower_ap(out)],
                        )
                    )

                def start_prev(prev, tagn):
                    """Denominator matmul + scalar reciprocal, emitted at the
                    top of the next window so the chain completes before the
                    normalize-copy at the window's end."""
                    pAb = prev[1]
                    d_ps = ps_z.tile([P, N], F32, tag="z", name=f"d_{tagn}")
                    for nh in range(2):
                        nc.tensor.matmul(
                            d_ps[:, nh * 512 : (nh + 1) * 512],
                            ones128[:],
                            pAb[:, nh * 512 : (nh + 1) * 512],
                            start=True,
                            stop=True,
                        )
                    R = r_pool.tile([P, N], F32, tag="r")
                    _scalar_recip(R[:], d_ps[:])
                    return R

                def unit_step(h, ch, Qth, Kth, prev):
                    """Scores+exp for unit (h, ch); attn@V + normalize for prev."""
                    chp = (1 - ch) if h < N_CROSS else ch  # kv channel
                    q0 = ch * N
                    m0 = chp * N
                    PT = pt_pool.tile([P, KO, N], BF, tag="pt")
                    S2 = s2_pool.tile([P, 4, N], BF, tag="s2")
                    zt_ps = None
                    if prev is not None:
                        pPT, pchp, ph = prev[0], prev[3], prev[2]
                        R = start_prev(prev, f"{h}_{ch}")
                        zt_ps = ps_z.tile([P, N], F32, tag="z", name=f"zt_{h}_{ch}")
                    for mi in range(KO):
                        ps2 = ps_a.tile([P, N], F32, tag="a", name=f"s_{h}_{ch}_{mi}")
                        for nh in range(2):
                            nc.tensor.matmul(
                                ps2[:, nh * 512 : (nh + 1) * 512],
                                Kth[:, m0 + mi * P : m0 + (mi + 1) * P],
                                Qth[:, q0 + nh * 512 : q0 + (nh + 1) * 512],
                                start=True,
                                stop=True,
                            )
                        nc.scalar.activation(PT[:, mi], ps2[:], EXP, scale=SCALE)
                        if mi % 2 == 1:
                            nc.vector.tensor_tensor(
                                S2[:, mi // 2], PT[:, mi - 1], PT[:, mi], ADD
                            )
                        if prev is not None:
                            vch = pchp * KO + mi
                            for nh in range(2):
                                nc.tensor.matmul(
                                    zt_ps[:, nh * 512 : (nh + 1) * 512],
                                    Vg[:, vch, ph, :],
                                    pPT[:, mi, nh * 512 : (nh + 1) * 512],
                                    start=(mi == 0),
                                    stop=(mi == KO - 1),
                                )
                    if prev is not None:
                        nc.vector.tensor_tensor(
                            Zt[:, prev[2], prev[4] : prev[4] + N],
                            zt_ps[:],
                            R[:],
                            MULT,
                        )
                    nc.vector.tensor_tensor(S2[:, 0], S2[:, 0], S2[:, 1], ADD)
                    nc.vector.tensor_tensor(S2[:, 2], S2[:, 2], S2[:, 3], ADD)
                    Ab = ab_pool.tile([P, N], BF, tag="ab")
                    nc.vector.tensor_tensor(Ab[:], S2[:, 0], S2[:, 2], ADD)
                    return (PT, Ab, h, chp, q0)

                def attnv_flush(prev):
                    pPT, pchp, ph = prev[0], prev[3], prev[2]
                    R = start_prev(prev, "flush")
                    zt_ps = ps_z.tile([P, N], F32, tag="z", name="zt_flush")
                    for mi in range(KO):
                        vch = pchp * KO + mi
                        for nh in range(2):
                            nc.tensor.matmul(
                                zt_ps[:, nh * 512 : (nh + 1) * 512],
                                Vg[:, vch, ph, :],
                                pPT[:, mi, nh * 512 : (nh + 1) * 512],
                                start=(mi == 0),
                                stop=(mi == KO - 1),
                            )
                    nc.vector.tensor_tensor(
                        Zt[:, ph, prev[4] : prev[4] + N], zt_ps[:], R[:], MULT
                    )

                # Q/K are projected one head AHEAD, interleaved between the
                # attention unit loops, so the scalar exp backlog and the
                # denominator chain drain during exp-independent PE work.
                Qcur = proj_head(0, Wq, bq_p, "qth")
                Kcur = proj_head(0, Wk, bk_p, "kth")
                prev = None
                Qnext = Knext = None
                for h in range(H):
                    prev = unit_step(h, 0, Qcur, Kcur, prev)
                    if h < H - 1:
                        Qnext = proj_head(h + 1, Wq, bq_p, "qth")
                    prev = unit_step(h, 1, Qcur, Kcur, prev)
                    if h < H - 1:
                        Knext = proj_head(h + 1, Wk, bk_p, "kth")
                        Qcur, Kcur = Qnext, Knext
                attnv_flush(prev)

            # ---- phase C: output projection ----
            with (
                tc.tile_pool(name="wo_pool", bufs=1) as wo_pool,
                tc.tile_pool(name="y_pool", bufs=2) as y_pool,
            ):
                Wo = wo_pool.tile([P, KO, D], BF)
                for q in range(4):
                    nc.gpsimd.dma_start(
                        Wo[:, 0, q * 256 : (q + 1) * 256],
                        wo_r[:, 0, q * 256 : (q + 1) * 256],
                    )
                for k in range(1, KO):
                    nc.gpsimd.dma_start(Wo[:, k, :], wo_r[:, k, :])
                for tci in range(TC):
                    pool, tag = (ps_a, "a") if tci % 2 == 0 else (ps_z, "z")
                    ps2 = pool.tile([P, D], F32, tag=tag, name=f"o_{tci}")
                    for k in range(KO):
                        for hf in range(2):
                            nc.tensor.matmul(
                                ps2[:, hf * 512 : (hf + 1) * 512],
                                Zt[:, k, tci * P : (tci + 1) * P],
                                Wo[:, k, hf * 512 : (hf + 1) * 512],
                                start=(k == 0),
                                stop=(k == KO - 1),
                            )
                    y = y_pool.tile([P, D], BF, tag="y")
                    nc.vector.tensor_tensor(y[:], ps2[:], bo_r[:], ADD)
                    dq = nc.sync if tci % 2 == 0 else nc.scalar
                    dq.dma_start(out_d[tci * P : (tci + 1) * P, :], y[:])
    return nc


def _get_program():
    if "nc" not in _CACHE:
        _legalize_install()
        _CACHE["nc"] = _build()
    return _CACHE["nc"]


def make_in_maps(inputs):
    x = np.asarray(inputs["x"], dtype=np.float32)
    bs2 = x.shape[0]
    n_cores = bs2 // 2
    bf = ml_dtypes.bfloat16

    weights = {
        name: np.ascontiguousarray(np.asarray(inputs[name], dtype=np.float32)).astype(
            bf
        )
        for name in ("Wq", "Wk", "Wv", "Wo")
    }
    biases = {
        name: np.ascontiguousarray(np.asarray(inputs[name], dtype=np.float32))
        for name in ("bq", "bk", "bv", "bo")
    }

    in_maps = []
    for c in range(n_cores):
        xt = np.ascontiguousarray(x[2 * c : 2 * c + 2].reshape(T, D).T).astype(bf)
        in_maps.append(
            {
                "xt": xt,
                "wq": weights["Wq"],
                "wk": weights["Wk"],
                "wv": weights["Wv"],
                "wo": weights["Wo"],
                "bq": biases["bq"],
                "bk": biases["bk"],
                "bv": biases["bv"],
                "bo": biases["bo"],
            }
        )
    return in_maps


def kernel(**inputs):
    bs2 = np.asarray(inputs["x"]).shape[0]
    n_cores = bs2 // 2
    in_maps = make_in_maps(inputs)
    nc = _get_program()
    res = run_bass_kernel_spmd(nc, in_maps, core_ids=list(range(n_cores)))
    out = np.empty((bs2, N, D), dtype=np.float32)
    for c in range(n_cores):
        out[2 * c : 2 * c + 2] = (
            res.results[c]["out"].astype(np.float32).reshape(2, N, D)
        )
    return out


# revision 25
# speedup vs baseline: 1.2761x; 1.0180x over previous
"""Cross-channel multi-head attention on 8 Trainium2 NeuronCores.

Sharding: data-parallel over the batch axis. bs2=16 sequences form bs=8
(batch, 2-channel) pairs; each core handles one pair fully locally
(cross-channel attention couples only the two channels of the same batch
element), so no collectives are needed.

Per core (T=2048 tokens = 2 channels x 1024 patches, D=1024, H=8 heads,
dk=128; heads 0..5 attend to the other channel's K/V, heads 6..7 to the
same channel):
  1. V = x @ Wv + bv in natural [T, D] layout (phase A), Qt/Kt = per-head
     [dk, T] projections (phase B) -- all matmul streams 512 wide so the
     PE's per-matmul LDWEIGHTS hides under the previous multiply.
  2. Per (head, channel) unit: S^T chunks = Kt-chunk^T x Qt (psum
     [128, 1024] spanning 2 banks, halves as separate matmul groups);
     P^T = exp(S^T/sqrt(dk)) via one [128,1024] scalar ACTIVATE per chunk.
  3. attn@V with V as the STATIONARY operand and P^T streaming 512 wide:
     Z^T[dk, n] accumulates directly in psum -- no PE transposes, no
     ones-column. Softmax denominators: DVE pair-add tree over the 8 P^T
     chunks (bf16), then an all-ones [128,128] stationary matmul whose
     output is the partition sum REPLICATED across all 128 partitions
     (f32 accumulate), DVE reciprocal psum->sbuf; the normalize then
     multiplies along the free dim in the psum->Zt copy.
  4. out = Zt-chunks^T @ Wo + bo, stored bf16 (host upcasts to f32).

All matmuls bf16 with f32 PSUM accumulation. Denominator tree in bf16
(values ~1e2..1e4, well within range; adds ~0.3% rel err, total ~0.8%
vs the 2e-2 gate). The host pre-transposes/casts x to bf16 [D, T] per
core and casts the weights to bf16.
"""

import sys

if "/opt/trn_rl_repo" not in sys.path:
    sys.path.insert(0, "/opt/trn_rl_repo")

import numpy as np
import ml_dtypes

import concourse.bass as bass
import concourse.bass_isa as bass_isa
import concourse.tile as tile
from concourse import mybir
from concourse.bass_utils import run_bass_kernel_spmd

# Walrus in this container rejects >1 wait condition on TPB_CTRL ops
# (Tile's kernel-tail drain carries one per active proc). Split them.
import os

_here = os.path.dirname(os.path.abspath(__file__))
if _here not in sys.path:
    sys.path.insert(0, _here)
try:
    import bir_legalize
except ImportError:  # graded in a bare dir: fall back to inline copy
    bir_legalize = None

N = 1024  # patches per channel
D = 1024
H = 8
DK = 128
N_CROSS = 6
T = 2 * N  # tokens per core (2 channels of one batch element)
P = 128
KO = D // P  # 8 outer chunks of the 1024-wide dims
TC = T // P  # 16 token chunks
BF = mybir.dt.bfloat16
F32 = mybir.dt.float32
SCALE = 1.0 / float(np.sqrt(DK))
EXP = mybir.ActivationFunctionType.Exp
ADD = mybir.AluOpType.add
MULT = mybir.AluOpType.mult

_CACHE = {}


def _legalize_install():
    if bir_legalize is not None:
        bir_legalize.install()
        return
    # Inline fallback (kernel.py must be self-contained when graded).
    import json
    import concourse.bass2jax as bass2jax
    from concourse.bass_utils import compile_bir_kernel as _orig

    if getattr(bass2jax.compile_bir_kernel, "_legalized", False):
        return

    OPCODE_MAX = {}
    SKIP = set()

    def _legalize(bir_json):
        d = json.loads(bir_json)
        changed = False
        for fn in d.get("functions", []):
            for bb in fn.get("blocks") or fn.get("basicblocks") or []:
                out = []
                for inst in bb.get("instructions", []):
                    sync = inst.get("sync_info") or {}
                    waits = sync.get("on_wait") or []
                    cap = OPCODE_MAX.get(inst.get("opcode"), 1)
                    if len(waits) > cap and inst.get("opcode") not in SKIP:
                        extra, keep = waits[:-cap], waits[-cap:]
                        for i, w in enumerate(extra):
                            out.append(
                                {
                                    "debug": inst.get("debug", 0),
                                    "engine": inst["engine"],
                                    "ins": [],
                                    "outs": [],
                                    "is_reset_sema": False,
                                    "name": f"{inst['name']}-sw{i}",
                                    "opcode": "Drain",
                                    "sync_info": {"on_update": [], "on_wait": [w]},
                                }
                            )
                        sync["on_wait"] = keep
                        inst["sync_info"] = sync
                        changed = True
                    out.append(inst)
                bb["instructions"] = out
        return json.dumps(d).encode() if changed else bir_json

    def compile_bir_kernel(bir_json, tmpdir, neff_name="file.neff"):
        return _orig(_legalize(bir_json), tmpdir, neff_name)

    compile_bir_kernel._legalized = True
    bass2jax.compile_bir_kernel = compile_bir_kernel


def _bcast_rows(ap, p):
    """Replicate a 1-D DRAM AP across p partitions (stride-0 partition dim)."""
    return bass.AP(tensor=ap.tensor, offset=ap.offset, ap=[[0, p], *ap.ap])


def _build():
    nc = bass.Bass()

    xt_d = nc.dram_tensor("xt", [D, T], BF, kind="ExternalInput").ap()
    wq_d = nc.dram_tensor("wq", [D, D], BF, kind="ExternalInput").ap()
    wk_d = nc.dram_tensor("wk", [D, D], BF, kind="ExternalInput").ap()
    wv_d = nc.dram_tensor("wv", [D, D], BF, kind="ExternalInput").ap()
    wo_d = nc.dram_tensor("wo", [D, D], BF, kind="ExternalInput").ap()
    bq_d = nc.dram_tensor("bq", [D], F32, kind="ExternalInput").ap()
    bk_d = nc.dram_tensor("bk", [D], F32, kind="ExternalInput").ap()
    bv_d = nc.dram_tensor("bv", [D], F32, kind="ExternalInput").ap()
    bo_d = nc.dram_tensor("bo", [D], F32, kind="ExternalInput").ap()
    out_d = nc.dram_tensor("out", [T, D], BF, kind="ExternalOutput").ap()

    xt_r = xt_d.rearrange("(o p) t -> p o t", p=P)
    wq_r = wq_d.rearrange("(o p) f -> p o f", p=P)
    wk_r = wk_d.rearrange("(o p) f -> p o f", p=P)
    wv_r = wv_d.rearrange("(o p) f -> p o f", p=P)
    wo_r = wo_d.rearrange("(o p) f -> p o f", p=P)

    with tile.TileContext(nc) as tc:
        with (
            tc.tile_pool(name="consts", bufs=1) as consts,
            tc.tile_pool(name="big", bufs=1) as big,
            tc.tile_pool(name="xt_w", bufs=1) as xt_w,
            tc.tile_pool(name="ps_a", bufs=4, space="PSUM") as ps_a,
            tc.tile_pool(name="ps_z", bufs=2, space="PSUM") as ps_z,
        ):
            Vg = big.tile([P, TC, H, DK], BF)  # V natural, per token-chunk/head
            Zt = big.tile([P, H, T], BF)  # attention out, [dout, T]
            bq_p = consts.tile([P, KO], F32)
            bk_p = consts.tile([P, KO], F32)
            bo_r = consts.tile([P, D], F32)

            # ---- phase A: V projection ----
            with tc.tile_pool(name="wv_pool", bufs=1) as wv_pool:
                Xt = xt_w.tile([P, KO, T], BF)
                Wq = xt_w.tile([P, KO, D], BF)
                Wk = xt_w.tile([P, KO, D], BF)
                Wv = wv_pool.tile([P, KO, D], BF)
                bv_r = wv_pool.tile([P, D], F32)

                # DMA issue order is the startup critical path. sync carries
                # the first-half Xt k-chunks (consumed k-ascending by batch
                # 0/1), scalar carries Wv then biases then Wq/Wk, gpsimd
                # carries the second-half Xt (batches 2/3, prefetched).
                nc.sync.dma_start(Xt[:, 0, 0:512], xt_r[:, 0, 0:512])
                nc.scalar.dma_start(Wv[:, 0, :], wv_r[:, 0, :])
                for k in range(1, KO):
                    nc.sync.dma_start(Xt[:, k, 0:512], xt_r[:, k, 0:512])
                    nc.scalar.dma_start(Wv[:, k, :], wv_r[:, k, :])
                for k in range(KO):
                    nc.sync.dma_start(Xt[:, k, 512:1024], xt_r[:, k, 512:1024])
                    nc.gpsimd.dma_start(Xt[:, k, 1024:2048], xt_r[:, k, 1024:2048])
                nc.scalar.dma_start(bq_p[:], bq_d.rearrange("(o p) -> p o", p=P))
                nc.scalar.dma_start(bk_p[:], bk_d.rearrange("(o p) -> p o", p=P))
                nc.scalar.dma_start(bv_r[:], _bcast_rows(bv_d, P))
                for k in range(KO):
                    nc.scalar.dma_start(Wq[:, k, :], wq_r[:, k, :])
                    nc.scalar.dma_start(Wk[:, k, :], wk_r[:, k, :])
                nc.scalar.dma_start(bo_r[:], _bcast_rows(bo_d, P))

                # Warm the PE HAM clock gate with throwaway accumulating
                # matmuls so the first real matmuls run at 2.4 GHz; sized to
                # roughly cover the first-input DMA window.
                warm_in = consts.tile([P, P], BF)
                nc.vector.memset(warm_in[:], 0.0)
                warm_rhs = consts.tile([P, 512], BF)
                nc.vector.memset(warm_rhs[:], 0.0)
                ones128 = consts.tile([P, P], BF)
                nc.vector.memset(ones128[:], 1.0)
                for g in range(2):
                    wps = ps_a.tile([P, 512], F32, tag="a", name=f"warm_{g}")
                    for k in range(8):
                        nc.tensor.matmul(
                            wps[:],
                            warm_in[:],
                            warm_rhs[:],
                            start=(k == 0),
                            stop=(k == 7),
                        )

                # V natural = Xt-chunk.T @ Wv, 4 two-bank psum groups in
                # flight (8 banks) so the PE has backlog while inputs stream.
                for base in range(0, TC, 4):
                    ztiles = [
                        ps_z.tile([P, D], F32, tag="z", name=f"vz_{base}_{t}")
                        for t in range(2)
                    ]
                    atiles = [
                        ps_a.tile([P, 512], F32, tag="a", name=f"va_{base}_{t}")
                        for t in range(4)
                    ]
                    for k in range(KO):
                        for t in range(2):
                            tci = base + t
                            for hh in range(2):
                                nc.tensor.matmul(
                                    ztiles[t][:, hh * 512 : (hh + 1) * 512],
                                    Xt[:, k, tci * P : (tci + 1) * P],
                                    Wv[:, k, hh * 512 : (hh + 1) * 512],
                                    start=(k == 0),
                                    stop=(k == KO - 1),
                                )
                        for t in range(4):
                            tci = base + 2 + t // 2
                            hh = t % 2
                            nc.tensor.matmul(
                                atiles[t][:],
                                Xt[:, k, tci * P : (tci + 1) * P],
                                Wv[:, k, hh * 512 : (hh + 1) * 512],
                                start=(k == 0),
                                stop=(k == KO - 1),
                            )
                    bv_h = bv_r.rearrange("p (h d) -> p h d", d=DK)
                    for t in range(2):
                        nc.vector.tensor_tensor(
                            Vg[:, base + t],
                            ztiles[t].rearrange("p (h d) -> p h d", d=DK),
                            bv_h,
                            ADD,
                        )
                    for t in range(4):
                        tci = base + 2 + t // 2
                        hh = t % 2
                        nc.vector.tensor_tensor(
                            Vg[:, tci, 4 * hh : 4 * hh + 4, :],
                            atiles[t].rearrange("p (h d) -> p h d", d=DK),
                            bv_h[:, 4 * hh : 4 * hh + 4, :],
                            ADD,
                        )

            # ---- phase B: per-head Q/K projection + attention ----
            with (
                tc.tile_pool(name="qk", bufs=2) as qk,
                tc.tile_pool(name="pt_pool", bufs=2) as pt_pool,
                tc.tile_pool(name="s2_pool", bufs=1) as s2_pool,
                tc.tile_pool(name="ab_pool", bufs=2) as ab_pool,
                tc.tile_pool(name="r_pool", bufs=2) as r_pool,
            ):

                def proj_head(h, w_sb, b_p, tagname):
                    dst = qk.tile([P, T], BF, tag=tagname)
                    for t2 in range(2):
                        ps2 = ps_z.tile([P, D], F32, tag="z", name=f"qk_{h}_{t2}")
                        for k in range(KO):
                            for hf in range(2):
                                nc.tensor.matmul(
                                    ps2[:, hf * 512 : (hf + 1) * 512],
                                    w_sb[:, k, h * P : (h + 1) * P],
                                    Xt[:, k, t2 * N + hf * 512 : t2 * N + (hf + 1) * 512],
                                    start=(k == 0),
                                    stop=(k == KO - 1),
                                )
                        nc.vector.tensor_tensor(
                            dst[:, t2 * N : (t2 + 1) * N],
                            ps2[:],
                            b_p[:, h : h + 1].to_broadcast((P, N)),
                            ADD,
                        )
                    return dst

                def _scalar_recip(out, in_):
                    # The scalar-engine Reciprocal table is plenty accurate
                    # for the softmax denominator (~1e-5 rel) and runs at
                    # ~1.4us per [128,1024] vs 6.5us on the DVE. bass's
                    # activation() refuses func=Reciprocal, so emit directly.
                    ins = [nc.scalar.lower_ap(in_)]
                    for arg in (0.0, 1.0, 0.0):
                        ins.append(mybir.ImmediateValue(dtype=F32, value=arg))
                    nc.scalar.add_instruction(
                        mybir.InstActivation(
                            name=nc.get_next_instruction_name(),
                            func=mybir.ActivationFunctionType.Reciprocal,
                            ins=ins,
                            outs=[nc.scalar.lower_ap(out)],
                        )
                    )

                def start_prev(prev, tagn):
                    """Denominator matmul + scalar reciprocal, emitted at the
                    top of the next window so the chain completes before the
                    normalize-copy at the window's end."""
                    pAb = prev[1]
                    d_ps = ps_z.tile([P, N], F32, tag="z", name=f"d_{tagn}")
                    for nh in range(2):
                        nc.tensor.matmul(
                            d_ps[:, nh * 512 : (nh + 1) * 512],
                            ones128[:],
                            pAb[:, nh * 512 : (nh + 1) * 512],
                            start=True,
                            stop=True,
                        )
                    R = r_pool.tile([P, N], F32, tag="r")
                    _scalar_recip(R[:], d_ps[:])
                    return R

                def unit_step(h, ch, Qth, Kth, prev):
                    """Scores+exp for unit (h, ch); attn@V + normalize for prev."""
                    chp = (1 - ch) if h < N_CROSS else ch  # kv channel
                    q0 = ch * N
                    m0 = chp * N
                    PT = pt_pool.tile([P, KO, N], BF, tag="pt")
                    S2 = s2_pool.tile([P, 4, N], BF, tag="s2")
                    zt_ps = None
                    if prev is not None:
                        pPT, pchp, ph = prev[0], prev[3], prev[2]
                        R = start_prev(prev, f"{h}_{ch}")
                        zt_ps = ps_z.tile([P, N], F32, tag="z", name=f"zt_{h}_{ch}")
                    for mi in range(KO):
                        for nh in range(2):
                            ps2 = ps_a.tile(
                                [P, 512], F32, tag="a", name=f"s_{h}_{ch}_{mi}_{nh}"
                            )
                            nc.tensor.matmul(
                                ps2[:],
                                Kth[:, m0 + mi * P : m0 + (mi + 1) * P],
                                Qth[:, q0 + nh * 512 : q0 + (nh + 1) * 512],
                                start=True,
                                stop=True,
                            )
                            nc.scalar.activation(
                                PT[:, mi, nh * 512 : (nh + 1) * 512],
                                ps2[:],
                                EXP,
                                scale=SCALE,
                            )
                        if mi % 2 == 1:
                            nc.vector.tensor_tensor(
                                S2[:, mi // 2], PT[:, mi - 1], PT[:, mi], ADD
                            )
                        if prev is not None:
                            vch = pchp * KO + mi
                            for nh in range(2):
                                nc.tensor.matmul(
                                    zt_ps[:, nh * 512 : (nh + 1) * 512],
                                    Vg[:, vch, ph, :],
                                    pPT[:, mi, nh * 512 : (nh + 1) * 512],
                                    start=(mi == 0),
                                    stop=(mi == KO - 1),
                                )
                    if prev is not None:
                        nc.vector.tensor_tensor(
                            Zt[:, prev[2], prev[4] : prev[4] + N],
                            zt_ps[:],
                            R[:],
                            MULT,
                        )
                    nc.vector.tensor_tensor(S2[:, 0], S2[:, 0], S2[:, 1], ADD)
                    nc.vector.tensor_tensor(S2[:, 2], S2[:, 2], S2[:, 3], ADD)
                    Ab = ab_pool.tile([P, N], BF, tag="ab")
                    nc.vector.tensor_tensor(Ab[:], S2[:, 0], S2[:, 2], ADD)
                    return (PT, Ab, h, chp, q0)

                def attnv_flush(prev):
                    pPT, pchp, ph = prev[0], prev[3], prev[2]
                    R = start_prev(prev, "flush")
                    zt_ps = ps_z.tile([P, N], F32, tag="z", name="zt_flush")
                    for mi in range(KO):
                        vch = pchp * KO + mi
                        for nh in range(2):
                            nc.tensor.matmul(
                                zt_ps[:, nh * 512 : (nh + 1) * 512],
                                Vg[:, vch, ph, :],
                                pPT[:, mi, nh * 512 : (nh + 1) * 512],
                                start=(mi == 0),
                                stop=(mi == KO - 1),
                            )
                    nc.vector.tensor_tensor(
                        Zt[:, ph, prev[4] : prev[4] + N], zt_ps[:], R[:], MULT
                    )

                # Q/K are projected one head AHEAD, interleaved between the
                # attention unit loops, so the scalar exp backlog and the
                # denominator chain drain during exp-independent PE work.
                Qcur = proj_head(0, Wq, bq_p, "qth")
                Kcur = proj_head(0, Wk, bk_p, "kth")
                prev = None
                Qnext = Knext = None
                for h in range(H):
                    prev = unit_step(h, 0, Qcur, Kcur, prev)
                    if h < H - 1:
                        Qnext = proj_head(h + 1, Wq, bq_p, "qth")
                    prev = unit_step(h, 1, Qcur, Kcur, prev)
                    if h < H - 1:
                        Knext = proj_head(h + 1, Wk, bk_p, "kth")
                        Qcur, Kcur = Qnext, Knext
                attnv_flush(prev)

            # ---- phase C: output projection ----
            with (
                tc.tile_pool(name="wo_pool", bufs=1) as wo_pool,
                tc.tile_pool(name="y_pool", bufs=2) as y_pool,
            ):
                Wo = wo_pool.tile([P, KO, D], BF)
                for q in range(4):
                    nc.gpsimd.dma_start(
                        Wo[:, 0, q * 256 : (q + 1) * 256],
                        wo_r[:, 0, q * 256 : (q + 1) * 256],
                    )
                for k in range(1, KO):
                    nc.gpsimd.dma_start(Wo[:, k, :], wo_r[:, k, :])
                for tci in range(TC):
                    ps2 = ps_z.tile([P, D], F32, tag="z", name=f"o_{tci}")
                    for k in range(KO):
                        for hf in range(2):
                            nc.tensor.matmul(
                                ps2[:, hf * 512 : (hf + 1) * 512],
                                Zt[:, k, tci * P : (tci + 1) * P],
                                Wo[:, k, hf * 512 : (hf + 1) * 512],
                                start=(k == 0),
                                stop=(k == KO - 1),
                            )
                    y = y_pool.tile([P, D], BF, tag="y")
                    nc.vector.tensor_tensor(y[:], ps2[:], bo_r[:], ADD)
                    dq = nc.sync if tci % 2 == 0 else nc.scalar
                    dq.dma_start(out_d[tci * P : (tci + 1) * P, :], y[:])
    return nc


def _get_program():
    if "nc" not in _CACHE:
        _legalize_install()
        _CACHE["nc"] = _build()
    return _CACHE["nc"]


def make_in_maps(inputs):
    x = np.asarray(inputs["x"], dtype=np.float32)
    bs2 = x.shape[0]
    n_cores = bs2 // 2
    bf = ml_dtypes.bfloat16

    weights = {
        name: np.ascontiguousarray(np.asarray(inputs[name], dtype=np.float32)).astype(
            bf
        )
        for name in ("Wq", "Wk", "Wv", "Wo")
    }
    biases = {
        name: np.ascontiguousarray(np.asarray(inputs[name], dtype=np.float32))
        for name in ("bq", "bk", "bv", "bo")
    }

    in_maps = []
    for c in range(n_cores):
        xt = np.ascontiguousarray(x[2 * c : 2 * c + 2].reshape(T, D).T).astype(bf)
        in_maps.append(
            {
                "xt": xt,
                "wq": weights["Wq"],
                "wk": weights["Wk"],
                "wv": weights["Wv"],
                "wo": weights["Wo"],
                "bq": biases["bq"],
                "bk": biases["bk"],
                "bv": biases["bv"],
                "bo": biases["bo"],
            }
        )
    return in_maps


def kernel(**inputs):
    bs2 = np.asarray(inputs["x"]).shape[0]
    n_cores = bs2 // 2
    in_maps = make_in_maps(inputs)
    nc = _get_program()
    res = run_bass_kernel_spmd(nc, in_maps, core_ids=list(range(n_cores)))
    out = np.empty((bs2, N, D), dtype=np.float32)
    for c in range(n_cores):
        out[2 * c : 2 * c + 2] = (
            res.results[c]["out"].astype(np.float32).reshape(2, N, D)
        )
    return out


# revision 26
# speedup vs baseline: 1.3379x; 1.0484x over previous
"""Cross-channel multi-head attention on 8 Trainium2 NeuronCores.

Sharding: data-parallel over the batch axis. bs2=16 sequences form bs=8
(batch, 2-channel) pairs; each core handles one pair fully locally
(cross-channel attention couples only the two channels of the same batch
element), so no collectives are needed.

Per core (T=2048 tokens = 2 channels x 1024 patches, D=1024, H=8 heads,
dk=128; heads 0..5 attend to the other channel's K/V, heads 6..7 to the
same channel):
  1. Qt = (x @ Wq + bq)^T and Kt likewise, in [D, T] layout (dk on
     partitions) -- exactly what the scores matmul wants as lhsT/rhs.
  2. V = x @ Wv + bv in natural [T, D] layout, stored per head with an
     extra ones column (softmax denominator trick).
  3. Per (head, channel): S^T[m,n] = Kt_h^T-slice x Qt_h-slice;
     P^T = exp(S^T / sqrt(dk)); Zt-slice = V_h-chunks contracted with
     P^T over m, landing directly in [dk, n] layout; denominators via
     DVE add-tree + gpsimd partition all-reduce, normalize fused into
     the PSUM->Zt copy.
  4. out = Zt^T-slices @ Wo + bo in natural [T, D] layout.

All matmuls in bf16 with f32 PSUM accumulation (~5e-3 rel err vs the f32
reference). The host pre-transposes/casts x to bf16 [D, T] per core and
casts the weights to bf16, so the device does no f32 transposes.
"""

import sys

if "/opt/trn_rl_repo" not in sys.path:
    sys.path.insert(0, "/opt/trn_rl_repo")

import numpy as np
import ml_dtypes

import concourse.bass as bass
import concourse.tile as tile
from concourse import mybir
from concourse.bass_utils import run_bass_kernel_spmd
from concourse.masks import make_identity
# Walrus in this container rejects >1 wait condition on TPB_CTRL ops
# (Tile's kernel-tail drain carries one per active proc). Split them.
import os

_here = os.path.dirname(os.path.abspath(__file__))
if _here not in sys.path:
    sys.path.insert(0, _here)
try:
    import bir_legalize
except ImportError:  # graded in a bare dir: fall back to inline copy
    bir_legalize = None

N = 1024  # patches per channel
D = 1024
H = 8
DK = 128
N_CROSS = 6
T = 2 * N  # tokens per core (2 channels of one batch element)
P = 128
KO = D // P  # 8 outer chunks of the 1024-wide dims
TC = T // P  # 16 token chunks
BF = mybir.dt.bfloat16
F32 = mybir.dt.float32
SCALE = 1.0 / float(np.sqrt(DK))

_CACHE = {}


def _legalize_install():
    if bir_legalize is not None:
        bir_legalize.install()
        return
    # Inline fallback (kernel.py must be self-contained when graded).
    import json
    import concourse.bass2jax as bass2jax
    from concourse.bass_utils import compile_bir_kernel as _orig

    if getattr(bass2jax.compile_bir_kernel, "_legalized", False):
        return

    OPCODE_MAX = {}
    SKIP = set()

    def _legalize(bir_json):
        d = json.loads(bir_json)
        changed = False
        for fn in d.get("functions", []):
            for bb in fn.get("blocks") or fn.get("basicblocks") or []:
                out = []
                for inst in bb.get("instructions", []):
                    sync = inst.get("sync_info") or {}
                    waits = sync.get("on_wait") or []
                    cap = OPCODE_MAX.get(inst.get("opcode"), 1)
                    if len(waits) > cap and inst.get("opcode") not in SKIP:
                        extra, keep = waits[:-cap], waits[-cap:]
                        for i, w in enumerate(extra):
                            out.append(
                                {
                                    "debug": inst.get("debug", 0),
                                    "engine": inst["engine"],
                                    "ins": [],
                                    "outs": [],
                                    "is_reset_sema": False,
                                    "name": f"{inst['name']}-sw{i}",
                                    "opcode": "Drain",
                                    "sync_info": {"on_update": [], "on_wait": [w]},
                                }
                            )
                        sync["on_wait"] = keep
                        inst["sync_info"] = sync
                        changed = True
                    out.append(inst)
                bb["instructions"] = out
        return json.dumps(d).encode() if changed else bir_json

    def compile_bir_kernel(bir_json, tmpdir, neff_name="file.neff"):
        return _orig(_legalize(bir_json), tmpdir, neff_name)

    compile_bir_kernel._legalized = True
    bass2jax.compile_bir_kernel = compile_bir_kernel


def _bcast_rows(ap, p):
    """Replicate a 1-D DRAM AP across p partitions (stride-0 partition dim)."""
    return bass.AP(tensor=ap.tensor, offset=ap.offset, ap=[[0, p], *ap.ap])


def _build():
    nc = bass.Bass()

    xt_d = nc.dram_tensor("xt", [D, T], BF, kind="ExternalInput").ap()
    wq_d = nc.dram_tensor("wq", [D, D], BF, kind="ExternalInput").ap()
    wk_d = nc.dram_tensor("wk", [D, D], BF, kind="ExternalInput").ap()
    wv_d = nc.dram_tensor("wv", [D, D], BF, kind="ExternalInput").ap()
    wo_d = nc.dram_tensor("wo", [D, D], BF, kind="ExternalInput").ap()
    bq_d = nc.dram_tensor("bq", [D], F32, kind="ExternalInput").ap()
    bk_d = nc.dram_tensor("bk", [D], F32, kind="ExternalInput").ap()
    bv_d = nc.dram_tensor("bv", [D], F32, kind="ExternalInput").ap()
    bo_d = nc.dram_tensor("bo", [D], F32, kind="ExternalInput").ap()
    out_d = nc.dram_tensor("out", [T, D], F32, kind="ExternalOutput").ap()

    with tile.TileContext(nc) as tc:
        with (
            tc.tile_pool(name="consts", bufs=1) as consts,
            tc.tile_pool(name="big", bufs=1) as big,
        ):
            ident = consts.tile([P, P], BF)
            make_identity(nc, ident)
            bq_p = consts.tile([P, KO], F32)
            nc.sync.dma_start(bq_p[:], bq_d.rearrange("(o p) -> p o", p=P))
            bk_p = consts.tile([P, KO], F32)
            nc.sync.dma_start(bk_p[:], bk_d.rearrange("(o p) -> p o", p=P))
            bv_r = consts.tile([P, D], F32)
            nc.sync.dma_start(bv_r[:], _bcast_rows(bv_d, P))
            bo_r = consts.tile([P, D], F32)
            nc.sync.dma_start(bo_r[:], _bcast_rows(bo_d, P))

            Vg = big.tile([P, TC, H, DK + 1], BF)  # natural V + ones col
            nc.vector.memset(Vg[:, :, :, DK : DK + 1], 1.0)
            Zt = big.tile([P, KO, T], BF)  # attention out, [dout, T]

            # ---- phases A (V proj) + B (QK proj fused with attention) ----
            with (
                tc.tile_pool(name="xt_w", bufs=1) as xt_w,
                tc.tile_pool(name="qk", bufs=2) as qk,
                tc.tile_pool(name="pt_pool", bufs=2) as pt_pool,
                tc.tile_pool(name="att_sm", bufs=4) as att_sm,
                tc.tile_pool(name="ps1", bufs=2, space="PSUM") as ps1,
                tc.tile_pool(name="ps_s", bufs=2, space="PSUM") as ps_s,
                tc.tile_pool(name="ps_z", bufs=2, space="PSUM") as ps_z,
                tc.tile_pool(name="ps_zt", bufs=2, space="PSUM") as ps_zt,
            ):
                # Warm the PE HAM clock gate with throwaway accumulating
                # matmul groups (dense, no psum rotation stalls) so the
                # first real matmuls run at 2.4 GHz instead of 1.2 GHz.
                warm_in = att_sm.tile([P, P], BF, tag="warm")
                nc.vector.memset(warm_in[:], 0.0)
                warm_rhs = att_sm.tile([P, 512], BF, tag="warm_rhs")
                nc.vector.memset(warm_rhs[:], 0.0)
                for g in range(2):
                    wps = ps_s.tile([P, 512], F32, tag="s")
                    for k in range(12):
                        nc.tensor.matmul(
                            wps[:],
                            warm_in[:],
                            warm_rhs[:],
                            start=(k == 0),
                            stop=(k == 11),
                        )

                # DMA: V-projection inputs (Xt, Wv) issue on sync, Wq/Wk on
                # gpsimd in parallel (descriptor issue is the startup
                # bottleneck). First k-chunks split fine across queues so the
                # first accumulation group can start ASAP.
                Xt = xt_w.tile([P, KO, T], BF)
                Wq = xt_w.tile([P, KO, D], BF)
                Wk = xt_w.tile([P, KO, D], BF)
                xt_r = xt_d.rearrange("(o p) t -> p o t", p=P)
                wq_r = wq_d.rearrange("(o p) f -> p o f", p=P)
                wk_r = wk_d.rearrange("(o p) f -> p o f", p=P)
                wv_r = wv_d.rearrange("(o p) f -> p o f", p=P)
                wv_ctx = tc.tile_pool(name="wv_pool", bufs=1)
                wv_pool = wv_ctx.__enter__()
                Wv = wv_pool.tile([P, KO, D], BF)
                for q in range(4):
                    nc.sync.dma_start(
                        Xt[:, 0, q * 512 : (q + 1) * 512],
                        xt_r[:, 0, q * 512 : (q + 1) * 512],
                    )
                nc.sync.dma_start(Wv[:, 0, :512], wv_r[:, 0, :512])
                nc.sync.dma_start(Wv[:, 0, 512:], wv_r[:, 0, 512:])
                for o in range(1, KO):
                    nc.sync.dma_start(Xt[:, o, :], xt_r[:, o, :])
                    nc.sync.dma_start(Wv[:, o, :], wv_r[:, o, :])
                for o in range(KO):
                    nc.gpsimd.dma_start(Wq[:, o, :], wq_r[:, o, :])
                    nc.gpsimd.dma_start(Wk[:, o, :], wk_r[:, o, :])

                # phase A: V natural = Xt-chunk.T @ Wv. Iterate k-OUTER
                # across 8 concurrent PSUM groups (borrowing every psum
                # pool's banks) so PE has a deep backlog while the input
                # DMAs are still streaming in k-chunk order.
                groups = [(tci, dh) for tci in range(TC) for dh in range(2)]
                gpools = [ps1, ps1, ps_s, ps_s, ps_z, ps_z, ps_zt, ps_zt]
                gtags = ["ps1", "ps1", "s", "s", "z", "z", "zt", "zt"]
                for base in range(0, len(groups), 8):
                    tiles = [
                        gpools[g].tile(
                            [P, 512], F32, tag=gtags[g], name=f"vps_{base}_{g}"
                        )
                        for g in range(8)
                    ]
                    for k in range(KO):
                        for g in range(8):
                            tci, dh = groups[base + g]
                            nc.tensor.matmul(
                                tiles[g][:],
                                Xt[:, k, tci * P : (tci + 1) * P],
                                Wv[:, k, dh * 512 : (dh + 1) * 512],
                                start=(k == 0),
                                stop=(k == KO - 1),
                            )
                    for g in range(8):
                        tci, dh = groups[base + g]
                        nc.vector.tensor_tensor(
                            Vg[:, tci, 4 * dh : 4 * dh + 4, :DK],
                            tiles[g].rearrange("p (h d) -> p h d", d=DK),
                            bv_r[:, dh * 512 : (dh + 1) * 512].rearrange(
                                "p (h d) -> p h d", d=DK
                            ),
                            mybir.AluOpType.add,
                        )

                wv_ctx.__exit__(None, None, None)

                # phase B: per head h: project Qt[h]/Kt[h], then the two
                # attention units, software-pipelined so attn@V of unit u-1
                # overlaps scores/exp of unit u (PT pool bufs=2).
                def proj_head(h, w_sb, b_p):
                    dst = qk.tile([P, T], BF, tag="qth" if w_sb is Wq else "kth")
                    for tt in range(T // 512):
                        ps = ps1.tile([P, 512], F32, tag="ps1")
                        for k in range(KO):
                            nc.tensor.matmul(
                                ps[:],
                                w_sb[:, k, h * P : (h + 1) * P],
                                Xt[:, k, tt * 512 : (tt + 1) * 512],
                                start=(k == 0),
                                stop=(k == KO - 1),
                            )
                        nc.vector.tensor_tensor(
                            dst[:, tt * 512 : (tt + 1) * 512],
                            ps[:],
                            b_p[:, h : h + 1].to_broadcast((P, 512)),
                            mybir.AluOpType.add,
                        )
                    return dst

                def scores_unit(h, ch, Qth, Kth):
                    chp = (1 - ch) if h < N_CROSS else ch  # kv channel
                    q0 = ch * N
                    m0 = chp * N
                    PT = pt_pool.tile([P, KO, N], BF, tag="pt")
                    for mi in range(KO):
                        for nh in range(2):
                            ps = ps_s.tile([P, 512], F32, tag="s")
                            nc.tensor.matmul(
                                ps[:],
                                Kth[:, m0 + mi * P : m0 + (mi + 1) * P],
                                Qth[:, q0 + nh * 512 : q0 + (nh + 1) * 512],
                                start=True,
                                stop=True,
                            )
                            nc.scalar.activation(
                                PT[:, mi, nh * 512 : (nh + 1) * 512],
                                ps[:],
                                mybir.ActivationFunctionType.Exp,
                                scale=SCALE,
                            )
                    return (PT, h, ch, chp, q0)

                def attnv_unit(state):
                    PT, h, ch, chp, q0 = state
                    for ni in range(KO):
                        psz = ps_z.tile([P, DK + 1], F32, tag="z")
                        for mi in range(KO):
                            nc.tensor.matmul(
                                psz[:],
                                PT[:, mi, ni * P : (ni + 1) * P],
                                Vg[:, chp * KO + mi, h, :],
                                start=(mi == 0),
                                stop=(mi == KO - 1),
                            )
                        r = att_sm.tile([P, 1], F32, tag="r")
                        nc.vector.reciprocal(r[:], psz[:, DK : DK + 1])
                        zn = att_sm.tile([P, DK], BF, tag="zn")
                        nc.vector.tensor_tensor(
                            zn[:],
                            psz[:, :DK],
                            r[:, 0:1].to_broadcast((P, DK)),
                            mybir.AluOpType.mult,
                        )
                        pzt = ps_zt.tile([P, P], BF, tag="zt")
                        nc.tensor.transpose(pzt[:], zn[:], ident[:])
                        nc.vector.tensor_copy(
                            Zt[:, h, q0 + ni * P : q0 + (ni + 1) * P], pzt[:]
                        )

                prev = None
                for h in range(H):
                    Qth = proj_head(h, Wq, bq_p)
                    Kth = proj_head(h, Wk, bk_p)
                    for ch in range(2):
                        cur = scores_unit(h, ch, Qth, Kth)
                        if prev is not None:
                            attnv_unit(prev)
                        prev = cur
                attnv_unit(prev)

            # ---- phase C: output projection ----
            with (
                tc.tile_pool(name="wo_pool", bufs=1) as wo_pool,
                tc.tile_pool(name="y_pool", bufs=4) as y_pool,
                tc.tile_pool(name="ps_y", bufs=4, space="PSUM") as ps_y,
            ):
                Wo = wo_pool.tile([P, KO, D], BF)
                wo_r = wo_d.rearrange("(o p) f -> p o f", p=P)
                for o in range(KO):
                    nc.sync.dma_start(Wo[:, o, :], wo_r[:, o, :])
                for tci in range(TC):
                    for dh in range(2):
                        ps = ps_y.tile([P, 512], F32, tag="y")
                        for k in range(KO):
                            nc.tensor.matmul(
                                ps[:],
                                Zt[:, k, tci * P : (tci + 1) * P],
                                Wo[:, k, dh * 512 : (dh + 1) * 512],
                                start=(k == 0),
                                stop=(k == KO - 1),
                            )
                        y = y_pool.tile([P, 512], F32, tag="y_sb")
                        nc.vector.tensor_tensor(
                            y[:],
                            ps[:],
                            bo_r[:, dh * 512 : (dh + 1) * 512],
                            mybir.AluOpType.add,
                        )
                        nc.sync.dma_start(
                            out_d[
                                tci * P : (tci + 1) * P,
                                dh * 512 : (dh + 1) * 512,
                            ],
                            y[:],
                        )
    return nc


def _get_program():
    if "nc" not in _CACHE:
        _legalize_install()
        _CACHE["nc"] = _build()
    return _CACHE["nc"]


def make_in_maps(inputs):
    x = np.asarray(inputs["x"], dtype=np.float32)
    bs2 = x.shape[0]
    n_cores = bs2 // 2
    bf = ml_dtypes.bfloat16

    weights = {
        name: np.ascontiguousarray(np.asarray(inputs[name], dtype=np.float32)).astype(
            bf
        )
        for name in ("Wq", "Wk", "Wv", "Wo")
    }
    biases = {
        name: np.ascontiguousarray(np.asarray(inputs[name], dtype=np.float32))
        for name in ("bq", "bk", "bv", "bo")
    }

    in_maps = []
    for c in range(n_cores):
        xt = np.ascontiguousarray(x[2 * c : 2 * c + 2].reshape(T, D).T).astype(bf)
        in_maps.append(
            {
                "xt": xt,
                "wq": weights["Wq"],
                "wk": weights["Wk"],
                "wv": weights["Wv"],
                "wo": weights["Wo"],
                "bq": biases["bq"],
                "bk": biases["bk"],
                "bv": biases["bv"],
                "bo": biases["bo"],
            }
        )
    return in_maps


def kernel(**inputs):
    bs2 = np.asarray(inputs["x"]).shape[0]
    n_cores = bs2 // 2
    in_maps = make_in_maps(inputs)
    nc = _get_program()
    res = run_bass_kernel_spmd(nc, in_maps, core_ids=list(range(n_cores)))
    out = np.empty((bs2, N, D), dtype=np.float32)
    for c in range(n_cores):
        out[2 * c : 2 * c + 2] = res.results[c]["out"].reshape(2, N, D)
    return out

